# revision 13
# baseline (speedup 1.0000x reference)
"""CRF forward (log partition) on 8 NeuronCores, data-parallel over batch.

Math: the forward recurrence runs in probability space: with E = exp(T) and
G_t = exp(emissions_t), alpha_{t+1} = logit_t + LSE_j(T + alpha_t) becomes the
linear recurrence P_{t+1} = G_t o (E @ P_t).

All normalization is folded into the DATA on the host: each active step's
emission row is pre-scaled by 1/m_t[b] with m_t[b] = sum_i G[b,t,i]*rowmean(E)_i
(a deterministic per-sequence scalar), which keeps the state O(1) in bf16 range
without any data-dependent renorm on device.  The log-scales are accumulated in
float64 host-side and added back at the end.

Variable lengths via an extra DONE label D per group (46 labels on device):
E'[D,:45] = E[STOP,:], E'[D,D] = 1.0 (exact in bf16), column D otherwise 0.
Active steps emit 0 for D so P[D] stays exactly 0; the absorb step at t=len[b]
emits onehot(D), capturing LSE_j(T[STOP,j]+alpha_j) -- the final answer -- into
P[D]; later steps emit onehot(D) again, multiplying P[D] by exactly 1.0.

Shrinking-width steps: sequences are dealt longest-first round-robin across the
16 (core, group) slots, and within each slot sorted descending into columns, so
column k's sequence dies no later than a STATIC schedule width n_t allows.  The
state lives in ONE in-place tile; step t only updates columns [0, n_t), so dead
columns keep their DONE value frozen.  Any sequence too long for its column
(impossible under the static margin for uniform lengths, but checked) is
computed exactly on the host instead.

Per-core critical path per step: one bf16 matmul [92,92]x[92,n_t] with the
stationary blockdiag(E'^T,E'^T) kept loaded in the PE array (standalone
ldweights + non-self-loading matmuls), then one DVE tensor_mul.
"""

import numpy as np
import ml_dtypes

import concourse.bacc as bacc
import concourse.bass as bass
import concourse.mybir as mybir
import concourse.tile as tile
from concourse.bass_utils import run_bass_kernel_spmd

L = 45
START = 43
STOP = 44
LD = 46                    # labels + DONE landing pad
DONE = 45
B = 1024
S = 512
NCORES = 8
BPC = B // NCORES          # 128 sequences per core
NG = 2                     # groups per core
WCOL = BPC // NG           # 64 columns per group
PR = NG * LD               # 92 partition rows for packed state
TSTEPS = S + 1             # +1 appended absorb step
NSLOTS = NCORES * NG       # 16 (core, group) slots

F32 = mybir.dt.float32
BF16 = mybir.dt.bfloat16
NP_BF16 = ml_dtypes.bfloat16

# Static shrinking-width schedule: step t in [1, 512] updates columns [0, n_t).
# n_1 == WCOL always, so the in-place state is fully written by step 1.
_T_ARR = np.arange(1, TSTEPS)
_N_SCHED = np.minimum(
    WCOL, np.maximum(4, np.ceil(WCOL * (TSTEPS - _T_ARR) / TSTEPS).astype(int) + 2)
)
assert _N_SCHED[0] == WCOL
# Column lifetime: last step that still updates column k.
_T_COL = np.array(
    [int((np.where(_N_SCHED > k)[0] + 1).max()) for k in range(WCOL)], np.int64
)
# Per-step g block widths (block 0 is the full-width init state) and offsets.
_BLK_W = np.concatenate([[WCOL], _N_SCHED])          # [TSTEPS]
_BLK_OFF = np.concatenate([[0], np.cumsum(_BLK_W)])  # [TSTEPS+1]
GCOLS = int(_BLK_OFF[-1])
# Chunk boundaries (step indices): tiny leading chunks for a fast start.
_CHUNK_STEPS = [0, 1, 9, 41] + list(np.linspace(41, TSTEPS, 7).astype(int)[1:])
NCHUNK = len(_CHUNK_STEPS) - 1


def _build_nc():
    # Bacc (not raw Bass): its legalization splits multi-sem waits into
    # standalone event-semaphore instructions, which walrus codegen requires.
    nc = bacc.Bacc("TRN2", target_bir_lowering=False, debug=False, num_devices=NCORES)
    g_dram = nc.dram_tensor("g", [PR, GCOLS], BF16, kind="ExternalInput")
    e2t_dram = nc.dram_tensor("e2t", [PR, PR], BF16, kind="ExternalInput")
    wout_dram = nc.dram_tensor("wout", [PR, WCOL], BF16, kind="ExternalOutput")

    with tile.TileContext(nc) as tc:
        with (
            tc.tile_pool(name="const", bufs=1) as const_pool,
            tc.tile_pool(name="gchunks", bufs=1) as g_pool,
            tc.tile_pool(name="state", bufs=1) as state_pool,
            tc.tile_pool(name="ps_s", bufs=3, space="PSUM") as ps_s,
        ):
            e2t = const_pool.tile([PR, PR], BF16, tag="e2t")
            nc.sync.dma_start(e2t[:], e2t_dram[:])

            # First three chunks go out on separate engine queues so their
            # DGE setups overlap (the chain starts as soon as chunk 0+1 land).
            dma_eng = [nc.gpsimd, nc.scalar] + [nc.sync] * NCHUNK
            gtiles = []
            for c in range(NCHUNK):
                c0 = int(_BLK_OFF[_CHUNK_STEPS[c]])
                c1 = int(_BLK_OFF[_CHUNK_STEPS[c + 1]])
                gt = g_pool.tile([PR, c1 - c0], BF16, tag=f"g{c}")
                dma_eng[c].dma_start(gt[:], g_dram[:, c0:c1])
                gtiles.append(gt)

            # Load blockdiag(E'^T, E'^T) into the PE array once; every step
            # matmul below reuses it (redundant auto-ldweights are stripped
            # after tile legalization below).
            nc.tensor.ldweights(e2t[:])

            # In-place state: step 1 is full width, so the state tile is
            # fully written by the first tensor_mul; the first matmul's
            # moving operand is the host-folded W_0 block of g directly.
            w_state = state_pool.tile([PR, WCOL], BF16, tag="w")

            # Scratch PSUM target for the keep-the-PE-clock-hot dummy matmuls.
            dummy_ps = ps_s.tile([PR, WCOL], F32, tag="dummy")

            chunk_of = np.searchsorted(_CHUNK_STEPS, np.arange(TSTEPS), "right") - 1
            for t in range(1, TSTEPS):
                n = int(_N_SCHED[t - 1])
                moving = gtiles[0][:, 0:WCOL] if t == 1 else w_state[:, 0:n]
                s_ps = ps_s.tile([PR, WCOL], F32, tag="s")
                nc.tensor.matmul(s_ps[:, 0:n], e2t[:], moving, start=True, stop=True)
                # Dummy matmul with no consumers: fills the PE idle window
                # while the DVE runs, keeping the PE power-state ramped.
                nc.tensor.matmul(
                    dummy_ps[:, 0:8], e2t[:], e2t[:, 0:8], start=True, stop=True
                )
                c = int(chunk_of[t])
                off = int(_BLK_OFF[t] - _BLK_OFF[_CHUNK_STEPS[c]])
                nc.vector.tensor_mul(
                    w_state[:, 0:n], gtiles[c][:, off : off + n], s_ps[:, 0:n]
                )

            nc.sync.dma_start(wout_dram[:], w_state[:])

    # Tile legalization splits every bf16 matmult into LDWEIGHTS + MATMULT.
    # All those loads are of the SAME stationary tile, so keep only the
    # first (the explicit one above) and drop the rest.  The auto-inserted
    # loads carry no semaphore waits/updates (all sync lives on the
    # matmults), so removal is sync-neutral.
    kept_first = False
    for blk in nc.main_func.blocks:
        for i in list(blk.instructions):
            if isinstance(i, mybir.InstLdweights):
                if not kept_first:
                    kept_first = True
                elif i.sync_info is None:
                    blk.instructions.remove(i)

    nc.compile()
    return nc


_NC_CACHE = {}


def _get_nc():
    if "nc" not in _NC_CACHE:
        _NC_CACHE["nc"] = _build_nc()
    return _NC_CACHE["nc"]


def _host_norm(logit_b, len_b, T):
    """Exact float64 log-space forward for one sequence (fallback path)."""
    NEG_INF = -10000.0
    alpha = np.full(L, NEG_INF)
    alpha[START] = 0.0
    for t in range(len_b):
        mat = T + alpha[None, :]
        mx = mat.max(axis=1)
        alpha = logit_b[t] + np.log(np.exp(mat - mx[:, None]).sum(axis=1)) + mx
    v = alpha + T[STOP]
    mx = v.max()
    return np.log(np.exp(v - mx).sum()) + mx


def _prep_inputs(logits, lens, transitions):
    """Host-side preprocessing: exp + absorb-rewrite + deterministic
    per-(seq,step) scaling + length-sorted packing.  Stashes the float64
    log-scale accumulator, the column permutation, and any host-fallback
    results for _postprocess."""
    logits = np.asarray(logits, np.float32)
    lens = np.asarray(lens, np.int64)
    T = np.asarray(transitions, np.float64)

    E = np.exp(T)                      # [45,45] float64
    erow = E.mean(axis=1)              # mean_j E[i,j], [45]

    Eg = np.zeros((LD, LD), np.float64)
    Eg[:L, :L] = E
    Eg[DONE, :L] = E[STOP, :]
    Eg[DONE, DONE] = 1.0
    e2t = np.zeros((PR, PR), np.float64)
    e2t[:LD, :LD] = Eg.T
    e2t[LD:, LD:] = Eg.T

    G = np.exp(logits.astype(np.float64))          # [B,S,45]

    t_idx = np.arange(S)[None, :]                  # [1,S]
    active = t_idx < lens[:, None]                 # [B,S]

    # Fold step 0 and normalize it exactly: W0 = G0*E[:,START], scale 1/sum.
    W0 = G[:, 0, :] * E[:, START][None, :]         # [B,45]
    m0 = W0.sum(axis=1)                            # [B]
    G[:, 0, :] = W0 / m0[:, None]

    # Active steps t>=1: scale by 1/m_t, m_t = sum_i G_t[i]*erow[i].
    m = G @ erow                                   # [B,S]
    scale_mask = active & (t_idx > 0)
    np.divide(G, m[:, :, None], out=G, where=scale_mask[:, :, None])

    # log-scale accumulator: z[b] = log m0 + sum_{1<=t<len} log m_t.
    logm = np.where(scale_mask, np.log(m), 0.0)
    z = np.log(m0) + logm.sum(axis=1)

    # 46-label emissions: D gets 0 while active, onehot(D) from t>=len on.
    G46 = np.zeros((B, TSTEPS, LD), np.float64)
    G46[:, :S, :L] = np.where(active[:, :, None], G, 0.0)
    done_from = t_idx >= lens[:, None]             # includes absorb step
    G46[:, :S, DONE] = np.where(done_from, 1.0, 0.0)
    G46[:, S, DONE] = 1.0                          # appended step

    # Deal longest-first round-robin across the 16 (core, group) slots.
    order = np.argsort(-lens, kind="stable")
    slots = np.empty((NSLOTS, WCOL), np.int64)
    for r, b in enumerate(order):
        slots[r % NSLOTS][r // NSLOTS] = b
    # Host fallback for any sequence outliving its column's static lifetime.
    host_norms = {}
    logits64 = logits.astype(np.float64)
    for s in range(NSLOTS):
        for k in range(WCOL):
            b = slots[s][k]
            if lens[b] > _T_COL[k]:
                host_norms[int(b)] = _host_norm(logits64[b], int(lens[b]), T)

    _NC_CACHE["z"] = z
    _NC_CACHE["slots"] = slots
    _NC_CACHE["host_norms"] = host_norms

    g16 = G46.astype(NP_BF16)
    e2t16 = e2t.astype(NP_BF16)
    in_maps = []
    for c in range(NCORES):
        g_in = np.zeros((PR, GCOLS), NP_BF16)
        for g in range(NG):
            seqs = slots[c * NG + g]               # [WCOL] original indices
            rows = slice(g * LD, (g + 1) * LD)
            # Per-step blocks: step t occupies cols [_BLK_OFF[t], +width).
            gc = g16[seqs]                         # [WCOL, TSTEPS, LD]
            for t in range(TSTEPS):
                w = int(_BLK_W[t])
                o = int(_BLK_OFF[t])
                g_in[rows, o : o + w] = gc[:w, t, :].T
        in_maps.append({"g": g_in, "e2t": e2t16})
    return in_maps


def _postprocess(results, lens, transitions):
    z = _NC_CACHE["z"]
    slots = _NC_CACHE["slots"]
    host_norms = _NC_CACHE["host_norms"]
    norm = np.empty(B, np.float64)
    for c in range(NCORES):
        wout = np.asarray(results[c]["wout"]).astype(np.float64)  # [PR, WCOL]
        for g in range(NG):
            seqs = slots[c * NG + g]
            pdone = wout[g * LD + DONE, :]
            norm[seqs] = np.log(pdone) + z[seqs]
    for b, v in host_norms.items():
        norm[b] = v
    return norm.astype(np.float32)


def kernel(logits, lens, transitions):
    nc = _get_nc()
    in_maps = _prep_inputs(logits, lens, transitions)
    res = run_bass_kernel_spmd(nc, in_maps, list(range(NCORES)))
    return _postprocess(res.results, lens, transitions)


# revision 16
# speedup vs baseline: 1.0191x; 1.0191x over previous
"""CRF forward (log partition) on 8 NeuronCores, data-parallel over batch.

Math: the forward recurrence runs in probability space: with E = exp(T) and
G_t = exp(emissions_t), alpha_{t+1} = logit_t + LSE_j(T + alpha_t) becomes the
linear recurrence P_{t+1} = G_t o (E @ P_t).

All normalization is folded into the DATA on the host: each active step's
emission row is pre-scaled by 1/m_t[b] with m_t[b] = sum_i G[b,t,i]*rowmean(E)_i
(a deterministic per-sequence scalar), which keeps the state O(1) in bf16 range
without any data-dependent renorm on device.  The log-scales are accumulated in
float64 host-side and added back at the end.

Variable lengths via an extra DONE label D per group (46 labels on device):
E'[D,:45] = E[STOP,:], E'[D,D] = 1.0 (exact in bf16), column D otherwise 0.
Active steps emit 0 for D so P[D] stays exactly 0; the absorb step at t=len[b]
emits onehot(D), capturing LSE_j(T[STOP,j]+alpha_j) -- the final answer -- into
P[D]; later steps emit onehot(D) again, multiplying P[D] by exactly 1.0.

Shrinking-width steps: sequences are dealt longest-first round-robin across the
16 (core, group) slots, and within each slot sorted descending into columns, so
column k's sequence dies no later than a STATIC schedule width n_t allows.  The
state lives in ONE in-place tile; step t only updates columns [0, n_t), so dead
columns keep their DONE value frozen.  Any sequence too long for its column
(impossible under the static margin for uniform lengths, but checked) is
computed exactly on the host instead.

Per-core critical path per step: one bf16 matmul [92,92]x[92,n_t] with the
stationary blockdiag(E'^T,E'^T) kept loaded in the PE array (standalone
ldweights + non-self-loading matmuls), then one DVE tensor_mul.
"""

import numpy as np
import ml_dtypes

import concourse.bacc as bacc
import concourse.bass as bass
import concourse.mybir as mybir
import concourse.tile as tile
from concourse.bass_utils import run_bass_kernel_spmd

L = 45
START = 43
STOP = 44
LD = 46                    # labels + DONE landing pad
DONE = 45
B = 1024
S = 512
NCORES = 8
BPC = B // NCORES          # 128 sequences per core
NG = 2                     # groups per core
WCOL = BPC // NG           # 64 columns per group
PR = NG * LD               # 92 partition rows for packed state
TSTEPS = S + 1             # +1 appended absorb step
NSLOTS = NCORES * NG       # 16 (core, group) slots

F32 = mybir.dt.float32
BF16 = mybir.dt.bfloat16
NP_BF16 = ml_dtypes.bfloat16

# Static shrinking-width schedule: step t in [1, 512] updates columns [0, n_t).
# n_1 == WCOL always, so the in-place state is fully written by step 1.
_T_ARR = np.arange(1, TSTEPS)
_N_SCHED = np.minimum(
    WCOL, np.maximum(4, np.ceil(WCOL * (TSTEPS - _T_ARR) / TSTEPS).astype(int) + 2)
)
assert _N_SCHED[0] == WCOL
# Column lifetime: last step that still updates column k.
_T_COL = np.array(
    [int((np.where(_N_SCHED > k)[0] + 1).max()) for k in range(WCOL)], np.int64
)
# Per-step g block widths (block 0 is the full-width init state) and offsets.
_BLK_W = np.concatenate([[WCOL], _N_SCHED])          # [TSTEPS]
_BLK_OFF = np.concatenate([[0], np.cumsum(_BLK_W)])  # [TSTEPS+1]
GCOLS = int(_BLK_OFF[-1])
# Chunk boundaries (step indices): tiny leading chunks for a fast start.
_CHUNK_STEPS = [0, 1, 9, 41] + list(np.linspace(41, TSTEPS, 7).astype(int)[1:])
NCHUNK = len(_CHUNK_STEPS) - 1


def _build_nc():
    # Bacc (not raw Bass): its legalization splits multi-sem waits into
    # standalone event-semaphore instructions, which walrus codegen requires.
    nc = bacc.Bacc("TRN2", target_bir_lowering=False, debug=False, num_devices=NCORES)
    # The stationary e2t matrix rides as the first PR columns of g, so one
    # DMA (and one semaphore) gates both the ldweights and the first matmul.
    g_dram = nc.dram_tensor("g", [PR, PR + GCOLS], BF16, kind="ExternalInput")
    wout_dram = nc.dram_tensor("wout", [PR, WCOL], BF16, kind="ExternalOutput")

    with tile.TileContext(nc) as tc:
        with (
            tc.tile_pool(name="gchunks", bufs=1) as g_pool,
            tc.tile_pool(name="state", bufs=1) as state_pool,
            tc.tile_pool(name="ps_s", bufs=3, space="PSUM") as ps_s,
        ):
            # Chunk 0 carries [e2t | W_0 init block]; later chunks carry the
            # per-step emission blocks.  First chunks go out on separate
            # engine queues so their DGE setups overlap.
            dma_eng = [nc.gpsimd, nc.scalar] + [nc.sync] * NCHUNK
            gtiles = []
            for c in range(NCHUNK):
                c0 = PR + int(_BLK_OFF[_CHUNK_STEPS[c]]) if c > 0 else 0
                c1 = PR + int(_BLK_OFF[_CHUNK_STEPS[c + 1]])
                gt = g_pool.tile([PR, c1 - c0], BF16, tag=f"g{c}")
                dma_eng[c].dma_start(gt[:], g_dram[:, c0:c1])
                gtiles.append(gt)

            e2t = gtiles[0][:, 0:PR]

            # Load blockdiag(E'^T, E'^T) into the PE array once; every step
            # matmul below reuses it (redundant auto-ldweights are stripped
            # after tile legalization below).
            nc.tensor.ldweights(e2t)

            # In-place state: step 1 is full width, so the state tile is
            # fully written by the first tensor_mul; the first matmul's
            # moving operand is the host-folded W_0 block of g directly.
            w_state = state_pool.tile([PR, WCOL], BF16, tag="w")

            chunk_of = np.searchsorted(_CHUNK_STEPS, np.arange(TSTEPS), "right") - 1
            for t in range(1, TSTEPS):
                n = int(_N_SCHED[t - 1])
                moving = (
                    gtiles[0][:, PR : PR + WCOL] if t == 1 else w_state[:, 0:n]
                )
                s_ps = ps_s.tile([PR, WCOL], F32, tag="s")
                nc.tensor.matmul(s_ps[:, 0:n], e2t, moving, start=True, stop=True)
                c = int(chunk_of[t])
                off = int(_BLK_OFF[t] - _BLK_OFF[_CHUNK_STEPS[c]])
                if c == 0:
                    off += PR
                nc.vector.tensor_mul(
                    w_state[:, 0:n], gtiles[c][:, off : off + n], s_ps[:, 0:n]
                )

            nc.sync.dma_start(wout_dram[:], w_state[:])

    # Tile legalization splits every bf16 matmult into LDWEIGHTS + MATMULT.
    # All those loads are of the SAME stationary tile, so keep only the
    # first (the explicit one above) and drop the rest.  The auto-inserted
    # loads carry no semaphore waits/updates (all sync lives on the
    # matmults), so removal is sync-neutral.
    kept_first = False
    for blk in nc.main_func.blocks:
        for i in list(blk.instructions):
            if isinstance(i, mybir.InstLdweights):
                if not kept_first:
                    kept_first = True
                elif i.sync_info is None:
                    blk.instructions.remove(i)

    nc.compile()
    return nc


_NC_CACHE = {}


def _get_nc():
    if "nc" not in _NC_CACHE:
        _NC_CACHE["nc"] = _build_nc()
    return _NC_CACHE["nc"]


def _host_norm(logit_b, len_b, T):
    """Exact float64 log-space forward for one sequence (fallback path)."""
    NEG_INF = -10000.0
    alpha = np.full(L, NEG_INF)
    alpha[START] = 0.0
    for t in range(len_b):
        mat = T + alpha[None, :]
        mx = mat.max(axis=1)
        alpha = logit_b[t] + np.log(np.exp(mat - mx[:, None]).sum(axis=1)) + mx
    v = alpha + T[STOP]
    mx = v.max()
    return np.log(np.exp(v - mx).sum()) + mx


def _prep_inputs(logits, lens, transitions):
    """Host-side preprocessing: exp + absorb-rewrite + deterministic
    per-(seq,step) scaling + length-sorted packing.  Stashes the float64
    log-scale accumulator, the column permutation, and any host-fallback
    results for _postprocess."""
    logits = np.asarray(logits, np.float32)
    lens = np.asarray(lens, np.int64)
    T = np.asarray(transitions, np.float64)

    E = np.exp(T)                      # [45,45] float64
    erow = E.mean(axis=1)              # mean_j E[i,j], [45]

    Eg = np.zeros((LD, LD), np.float64)
    Eg[:L, :L] = E
    Eg[DONE, :L] = E[STOP, :]
    Eg[DONE, DONE] = 1.0
    e2t = np.zeros((PR, PR), np.float64)
    e2t[:LD, :LD] = Eg.T
    e2t[LD:, LD:] = Eg.T

    G = np.exp(logits.astype(np.float64))          # [B,S,45]

    t_idx = np.arange(S)[None, :]                  # [1,S]
    active = t_idx < lens[:, None]                 # [B,S]

    # Fold step 0 and normalize it exactly: W0 = G0*E[:,START], scale 1/sum.
    W0 = G[:, 0, :] * E[:, START][None, :]         # [B,45]
    m0 = W0.sum(axis=1)                            # [B]
    G[:, 0, :] = W0 / m0[:, None]

    # Active steps t>=1: scale by 1/m_t, m_t = sum_i G_t[i]*erow[i].
    m = G @ erow                                   # [B,S]
    scale_mask = active & (t_idx > 0)
    np.divide(G, m[:, :, None], out=G, where=scale_mask[:, :, None])

    # log-scale accumulator: z[b] = log m0 + sum_{1<=t<len} log m_t.
    logm = np.where(scale_mask, np.log(m), 0.0)
    z = np.log(m0) + logm.sum(axis=1)

    # 46-label emissions: D gets 0 while active, onehot(D) from t>=len on.
    G46 = np.zeros((B, TSTEPS, LD), np.float64)
    G46[:, :S, :L] = np.where(active[:, :, None], G, 0.0)
    done_from = t_idx >= lens[:, None]             # includes absorb step
    G46[:, :S, DONE] = np.where(done_from, 1.0, 0.0)
    G46[:, S, DONE] = 1.0                          # appended step

    # Deal longest-first round-robin across the 16 (core, group) slots.
    order = np.argsort(-lens, kind="stable")
    slots = np.empty((NSLOTS, WCOL), np.int64)
    for r, b in enumerate(order):
        slots[r % NSLOTS][r // NSLOTS] = b
    # Host fallback for any sequence outliving its column's static lifetime.
    host_norms = {}
    logits64 = logits.astype(np.float64)
    for s in range(NSLOTS):
        for k in range(WCOL):
            b = slots[s][k]
            if lens[b] > _T_COL[k]:
                host_norms[int(b)] = _host_norm(logits64[b], int(lens[b]), T)

    _NC_CACHE["z"] = z
    _NC_CACHE["slots"] = slots
    _NC_CACHE["host_norms"] = host_norms

    g16 = G46.astype(NP_BF16)
    e2t16 = e2t.astype(NP_BF16)
    in_maps = []
    for c in range(NCORES):
        g_in = np.zeros((PR, PR + GCOLS), NP_BF16)
        g_in[:, :PR] = e2t16
        for g in range(NG):
            seqs = slots[c * NG + g]               # [WCOL] original indices
            rows = slice(g * LD, (g + 1) * LD)
            # Per-step blocks: step t occupies cols PR + [_BLK_OFF[t], +width).
            gc = g16[seqs]                         # [WCOL, TSTEPS, LD]
            for t in range(TSTEPS):
                w = int(_BLK_W[t])
                o = PR + int(_BLK_OFF[t])
                g_in[rows, o : o + w] = gc[:w, t, :].T
        in_maps.append({"g": g_in})
    return in_maps


def _postprocess(results, lens, transitions):
    z = _NC_CACHE["z"]
    slots = _NC_CACHE["slots"]
    host_norms = _NC_CACHE["host_norms"]
    norm = np.empty(B, np.float64)
    for c in range(NCORES):
        wout = np.asarray(results[c]["wout"]).astype(np.float64)  # [PR, WCOL]
        for g in range(NG):
            seqs = slots[c * NG + g]
            pdone = wout[g * LD + DONE, :]
            norm[seqs] = np.log(pdone) + z[seqs]
    for b, v in host_norms.items():
        norm[b] = v
    return norm.astype(np.float32)


def kernel(logits, lens, transitions):
    nc = _get_nc()
    in_maps = _prep_inputs(logits, lens, transitions)
    res = run_bass_kernel_spmd(nc, in_maps, list(range(NCORES)))
    return _postprocess(res.results, lens, transitions)


# revision 17
# speedup vs baseline: 1.0266x; 1.0074x over previous
"""CRF forward (log partition) on 8 NeuronCores, data-parallel over batch.

Math: the forward recurrence runs in probability space: with E = exp(T) and
G_t = exp(emissions_t), alpha_{t+1} = logit_t + LSE_j(T + alpha_t) becomes the
linear recurrence P_{t+1} = G_t o (E @ P_t).

All normalization is folded into the DATA on the host: each active step's
emission row is pre-scaled by 1/m_t[b] with m_t[b] = sum_i G[b,t,i]*rowmean(E)_i
(a deterministic per-sequence scalar), which keeps the state O(1) in bf16 range
without any data-dependent renorm on device.  The log-scales are accumulated in
float64 host-side and added back at the end.

Variable lengths via an extra DONE label D per group (46 labels on device):
E'[D,:45] = E[STOP,:], E'[D,D] = 1.0 (exact in bf16), column D otherwise 0.
Active steps emit 0 for D so P[D] stays exactly 0; the absorb step at t=len[b]
emits onehot(D), capturing LSE_j(T[STOP,j]+alpha_j) -- the final answer -- into
P[D]; later steps emit onehot(D) again, multiplying P[D] by exactly 1.0.

Shrinking-width steps: sequences are dealt longest-first round-robin across the
16 (core, group) slots, and within each slot sorted descending into columns, so
column k's sequence dies no later than a STATIC schedule width n_t allows.  The
state lives in ONE in-place tile; step t only updates columns [0, n_t), so dead
columns keep their DONE value frozen.  Any sequence too long for its column
(impossible under the static margin for uniform lengths, but checked) is
computed exactly on the host instead.

Per-core critical path per step: one bf16 matmul [92,92]x[92,n_t] with the
stationary blockdiag(E'^T,E'^T) kept loaded in the PE array (standalone
ldweights + non-self-loading matmuls), then one DVE tensor_mul.
"""

import numpy as np
import ml_dtypes

import concourse.bacc as bacc
import concourse.bass as bass
import concourse.mybir as mybir
import concourse.tile as tile
from concourse.bass_utils import run_bass_kernel_spmd

L = 45
START = 43
STOP = 44
LD = 46                    # labels + DONE landing pad
DONE = 45
B = 1024
S = 512
NCORES = 8
BPC = B // NCORES          # 128 sequences per core
NG = 2                     # groups per core
WCOL = BPC // NG           # 64 columns per group
PR = NG * LD               # 92 partition rows for packed state
TSTEPS = S + 1             # +1 appended absorb step
NSLOTS = NCORES * NG       # 16 (core, group) slots

F32 = mybir.dt.float32
BF16 = mybir.dt.bfloat16
NP_BF16 = ml_dtypes.bfloat16

# Static shrinking-width schedule: step t in [1, 512] updates columns [0, n_t).
# n_1 == WCOL always, so the in-place state is fully written by step 1.
_T_ARR = np.arange(1, TSTEPS)
_N_SCHED = np.minimum(
    WCOL, np.maximum(4, np.ceil(WCOL * (TSTEPS - _T_ARR) / TSTEPS).astype(int) + 2)
)
assert _N_SCHED[0] == WCOL
# Column lifetime: last step that still updates column k.
_T_COL = np.array(
    [int((np.where(_N_SCHED > k)[0] + 1).max()) for k in range(WCOL)], np.int64
)
# Per-step g block widths (block 0 is the full-width init state) and offsets.
_BLK_W = np.concatenate([[WCOL], _N_SCHED])          # [TSTEPS]
_BLK_OFF = np.concatenate([[0], np.cumsum(_BLK_W)])  # [TSTEPS+1]
GCOLS = int(_BLK_OFF[-1])
# Chunk boundaries (step indices): tiny leading chunks for a fast start.
_CHUNK_STEPS = [0, 1, 9, 41] + list(np.linspace(41, TSTEPS, 7).astype(int)[1:])
NCHUNK = len(_CHUNK_STEPS) - 1


def _build_nc():
    # Bacc (not raw Bass): its legalization splits multi-sem waits into
    # standalone event-semaphore instructions, which walrus codegen requires.
    nc = bacc.Bacc("TRN2", target_bir_lowering=False, debug=False, num_devices=NCORES)
    # The stationary e2t matrix rides as the first PR columns of g, so one
    # DMA (and one semaphore) gates both the ldweights and the first matmul.
    g_dram = nc.dram_tensor("g", [PR, PR + GCOLS], BF16, kind="ExternalInput")
    wout_dram = nc.dram_tensor("wout", [PR, WCOL], BF16, kind="ExternalOutput")

    with tile.TileContext(nc) as tc:
        with (
            tc.tile_pool(name="gchunks", bufs=1) as g_pool,
            tc.tile_pool(name="state", bufs=1) as state_pool,
            tc.tile_pool(name="ps_s", bufs=3, space="PSUM") as ps_s,
        ):
            # Chunk 0 carries [e2t | W_0 init block]; later chunks carry the
            # per-step emission blocks.  First chunks go out on separate
            # engine queues so their DGE setups overlap.
            dma_eng = [nc.sync, nc.gpsimd, nc.scalar] + [nc.sync] * NCHUNK
            gtiles = []
            for c in range(NCHUNK):
                c0 = PR + int(_BLK_OFF[_CHUNK_STEPS[c]]) if c > 0 else 0
                c1 = PR + int(_BLK_OFF[_CHUNK_STEPS[c + 1]])
                gt = g_pool.tile([PR, c1 - c0], BF16, tag=f"g{c}")
                dma_eng[c].dma_start(gt[:], g_dram[:, c0:c1])
                gtiles.append(gt)

            e2t = gtiles[0][:, 0:PR]

            # Load blockdiag(E'^T, E'^T) into the PE array once; every step
            # matmul below reuses it (redundant auto-ldweights are stripped
            # after tile legalization below).
            nc.tensor.ldweights(e2t)

            # In-place state: step 1 is full width, so the state tile is
            # fully written by the first tensor_mul; the first matmul's
            # moving operand is the host-folded W_0 block of g directly.
            w_state = state_pool.tile([PR, WCOL], BF16, tag="w")

            chunk_of = np.searchsorted(_CHUNK_STEPS, np.arange(TSTEPS), "right") - 1
            for t in range(1, TSTEPS):
                n = int(_N_SCHED[t - 1])
                moving = (
                    gtiles[0][:, PR : PR + WCOL] if t == 1 else w_state[:, 0:n]
                )
                s_ps = ps_s.tile([PR, WCOL], F32, tag="s")
                nc.tensor.matmul(s_ps[:, 0:n], e2t, moving, start=True, stop=True)
                c = int(chunk_of[t])
                off = int(_BLK_OFF[t] - _BLK_OFF[_CHUNK_STEPS[c]])
                if c == 0:
                    off += PR
                nc.vector.tensor_mul(
                    w_state[:, 0:n], gtiles[c][:, off : off + n], s_ps[:, 0:n]
                )

            nc.sync.dma_start(wout_dram[:], w_state[:])

    # Tile legalization splits every bf16 matmult into LDWEIGHTS + MATMULT.
    # All those loads are of the SAME stationary tile, so keep only the
    # first (the explicit one above) and drop the rest.  The auto-inserted
    # loads carry no semaphore waits/updates (all sync lives on the
    # matmults), so removal is sync-neutral.
    kept_first = False
    for blk in nc.main_func.blocks:
        for i in list(blk.instructions):
            if isinstance(i, mybir.InstLdweights):
                if not kept_first:
                    kept_first = True
                elif i.sync_info is None:
                    blk.instructions.remove(i)

    nc.compile()
    return nc


_NC_CACHE = {}


def _get_nc():
    if "nc" not in _NC_CACHE:
        _NC_CACHE["nc"] = _build_nc()
    return _NC_CACHE["nc"]


def _host_norm(logit_b, len_b, T):
    """Exact float64 log-space forward for one sequence (fallback path)."""
    NEG_INF = -10000.0
    alpha = np.full(L, NEG_INF)
    alpha[START] = 0.0
    for t in range(len_b):
        mat = T + alpha[None, :]
        mx = mat.max(axis=1)
        alpha = logit_b[t] + np.log(np.exp(mat - mx[:, None]).sum(axis=1)) + mx
    v = alpha + T[STOP]
    mx = v.max()
    return np.log(np.exp(v - mx).sum()) + mx


def _prep_inputs(logits, lens, transitions):
    """Host-side preprocessing: exp + absorb-rewrite + deterministic
    per-(seq,step) scaling + length-sorted packing.  Stashes the float64
    log-scale accumulator, the column permutation, and any host-fallback
    results for _postprocess."""
    logits = np.asarray(logits, np.float32)
    lens = np.asarray(lens, np.int64)
    T = np.asarray(transitions, np.float64)

    E = np.exp(T)                      # [45,45] float64
    erow = E.mean(axis=1)              # mean_j E[i,j], [45]

    Eg = np.zeros((LD, LD), np.float64)
    Eg[:L, :L] = E
    Eg[DONE, :L] = E[STOP, :]
    Eg[DONE, DONE] = 1.0
    e2t = np.zeros((PR, PR), np.float64)
    e2t[:LD, :LD] = Eg.T
    e2t[LD:, LD:] = Eg.T

    G = np.exp(logits.astype(np.float64))          # [B,S,45]

    t_idx = np.arange(S)[None, :]                  # [1,S]
    active = t_idx < lens[:, None]                 # [B,S]

    # Fold step 0 and normalize it exactly: W0 = G0*E[:,START], scale 1/sum.
    W0 = G[:, 0, :] * E[:, START][None, :]         # [B,45]
    m0 = W0.sum(axis=1)                            # [B]
    G[:, 0, :] = W0 / m0[:, None]

    # Active steps t>=1: scale by 1/m_t, m_t = sum_i G_t[i]*erow[i].
    m = G @ erow                                   # [B,S]
    scale_mask = active & (t_idx > 0)
    np.divide(G, m[:, :, None], out=G, where=scale_mask[:, :, None])

    # log-scale accumulator: z[b] = log m0 + sum_{1<=t<len} log m_t.
    logm = np.where(scale_mask, np.log(m), 0.0)
    z = np.log(m0) + logm.sum(axis=1)

    # 46-label emissions: D gets 0 while active, onehot(D) from t>=len on.
    G46 = np.zeros((B, TSTEPS, LD), np.float64)
    G46[:, :S, :L] = np.where(active[:, :, None], G, 0.0)
    done_from = t_idx >= lens[:, None]             # includes absorb step
    G46[:, :S, DONE] = np.where(done_from, 1.0, 0.0)
    G46[:, S, DONE] = 1.0                          # appended step

    # Deal longest-first round-robin across the 16 (core, group) slots.
    order = np.argsort(-lens, kind="stable")
    slots = np.empty((NSLOTS, WCOL), np.int64)
    for r, b in enumerate(order):
        slots[r % NSLOTS][r // NSLOTS] = b
    # Host fallback for any sequence outliving its column's static lifetime.
    host_norms = {}
    logits64 = logits.astype(np.float64)
    for s in range(NSLOTS):
        for k in range(WCOL):
            b = slots[s][k]
            if lens[b] > _T_COL[k]:
                host_norms[int(b)] = _host_norm(logits64[b], int(lens[b]), T)

    _NC_CACHE["z"] = z
    _NC_CACHE["slots"] = slots
    _NC_CACHE["host_norms"] = host_norms

    g16 = G46.astype(NP_BF16)
    e2t16 = e2t.astype(NP_BF16)
    in_maps = []
    for c in range(NCORES):
        g_in = np.zeros((PR, PR + GCOLS), NP_BF16)
        g_in[:, :PR] = e2t16
        for g in range(NG):
            seqs = slots[c * NG + g]               # [WCOL] original indices
            rows = slice(g * LD, (g + 1) * LD)
            # Per-step blocks: step t occupies cols PR + [_BLK_OFF[t], +width).
            gc = g16[seqs]                         # [WCOL, TSTEPS, LD]
            for t in range(TSTEPS):
                w = int(_BLK_W[t])
                o = PR + int(_BLK_OFF[t])
                g_in[rows, o : o + w] = gc[:w, t, :].T
        in_maps.append({"g": g_in})
    return in_maps


def _postprocess(results, lens, transitions):
    z = _NC_CACHE["z"]
    slots = _NC_CACHE["slots"]
    host_norms = _NC_CACHE["host_norms"]
    norm = np.empty(B, np.float64)
    for c in range(NCORES):
        wout = np.asarray(results[c]["wout"]).astype(np.float64)  # [PR, WCOL]
        for g in range(NG):
            seqs = slots[c * NG + g]
            pdone = wout[g * LD + DONE, :]
            norm[seqs] = np.log(pdone) + z[seqs]
    for b, v in host_norms.items():
        norm[b] = v
    return norm.astype(np.float32)


def kernel(logits, lens, transitions):
    nc = _get_nc()
    in_maps = _prep_inputs(logits, lens, transitions)
    res = run_bass_kernel_spmd(nc, in_maps, list(range(NCORES)))
    return _postprocess(res.results, lens, transitions)


# revision 20
# speedup vs baseline: 1.0288x; 1.0021x over previous
"""CRF forward (log partition) on 8 NeuronCores, data-parallel over batch.

Math: the forward recurrence runs in probability space: with E = exp(T) and
G_t = exp(emissions_t), alpha_{t+1} = logit_t + LSE_j(T + alpha_t) becomes the
linear recurrence P_{t+1} = G_t o (E @ P_t).

All normalization is folded into the DATA on the host: each active step's
emission row is pre-scaled by 1/m_t[b] with m_t[b] = sum_i G[b,t,i]*rowmean(E)_i
(a deterministic per-sequence scalar), which keeps the state O(1) in bf16 range
without any data-dependent renorm on device.  The log-scales are accumulated in
float64 host-side and added back at the end.

Variable lengths via an extra DONE label D per group (46 labels on device):
E'[D,:45] = E[STOP,:], E'[D,D] = 1.0 (exact in bf16), column D otherwise 0.
Active steps emit 0 for D so P[D] stays exactly 0; the absorb step at t=len[b]
emits onehot(D), capturing LSE_j(T[STOP,j]+alpha_j) -- the final answer -- into
P[D]; later steps emit onehot(D) again, multiplying P[D] by exactly 1.0.

Shrinking-width steps: sequences are dealt longest-first round-robin across the
16 (core, group) slots, and within each slot sorted descending into columns, so
column k's sequence dies no later than a STATIC schedule width n_t allows.  The
state lives in ONE in-place tile; step t only updates columns [0, n_t), so dead
columns keep their DONE value frozen.  Any sequence too long for its column
(impossible under the static margin for uniform lengths, but checked) is
computed exactly on the host instead.

Per-core critical path per step: one bf16 matmul [92,92]x[92,n_t] with the
stationary blockdiag(E'^T,E'^T) kept loaded in the PE array (standalone
ldweights + non-self-loading matmuls), then one DVE tensor_mul.
"""

import numpy as np
import ml_dtypes

import concourse.bacc as bacc
import concourse.bass as bass
import concourse.mybir as mybir
import concourse.tile as tile
from concourse.bass_utils import run_bass_kernel_spmd

L = 45
START = 43
STOP = 44
LD = 46                    # labels + DONE landing pad
DONE = 45
B = 1024
S = 512
NCORES = 8
BPC = B // NCORES          # 128 sequences per core
NG = 2                     # groups per core
WCOL = BPC // NG           # 64 columns per group
PR = NG * LD               # 92 partition rows for packed state
TSTEPS = S + 1             # +1 appended absorb step
NSLOTS = NCORES * NG       # 16 (core, group) slots

F32 = mybir.dt.float32
BF16 = mybir.dt.bfloat16
NP_BF16 = ml_dtypes.bfloat16

# Static shrinking-width schedule: step t in [1, 512] updates columns [0, n_t).
# n_1 == WCOL always, so the in-place state is fully written by step 1.
_T_ARR = np.arange(1, TSTEPS)
_N_SCHED = np.minimum(
    WCOL, np.maximum(4, np.ceil(WCOL * (TSTEPS - _T_ARR) / TSTEPS).astype(int) + 2)
)
assert _N_SCHED[0] == WCOL
# Column lifetime: last step that still updates column k.
_T_COL = np.array(
    [int((np.where(_N_SCHED > k)[0] + 1).max()) for k in range(WCOL)], np.int64
)
# Per-step g block widths (block 0 is the full-width init state) and offsets.
_BLK_W = np.concatenate([[WCOL], _N_SCHED])          # [TSTEPS]
_BLK_OFF = np.concatenate([[0], np.cumsum(_BLK_W)])  # [TSTEPS+1]
GCOLS = int(_BLK_OFF[-1])
# Chunk boundaries (step indices): tiny leading chunks for a fast start; the
# later chunks rotate through a small pool so their DMAs trail compute.
_CHUNK_STEPS = [0, 1, 9, 41] + list(np.linspace(41, TSTEPS, 9).astype(int)[1:])
NCHUNK = len(_CHUNK_STEPS) - 1
_NEAGER = 3                      # chunks 0..2 load immediately, own tags
_ROT_W = int(
    max(
        _BLK_OFF[_CHUNK_STEPS[c + 1]] - _BLK_OFF[_CHUNK_STEPS[c]]
        for c in range(_NEAGER, NCHUNK)
    )
)


def _build_nc():
    # Bacc (not raw Bass): its legalization splits multi-sem waits into
    # standalone event-semaphore instructions, which walrus codegen requires.
    nc = bacc.Bacc("TRN2", target_bir_lowering=False, debug=False, num_devices=NCORES)
    # The stationary e2t matrix rides as the first PR columns of g, so one
    # DMA (and one semaphore) gates both the ldweights and the first matmul.
    g_dram = nc.dram_tensor("g", [PR, PR + GCOLS], BF16, kind="ExternalInput")
    wout_dram = nc.dram_tensor("wout", [PR, WCOL], BF16, kind="ExternalOutput")

    with tile.TileContext(nc) as tc:
        with (
            tc.tile_pool(name="geager", bufs=1) as ge_pool,
            tc.tile_pool(name="grot", bufs=2) as gr_pool,
            tc.tile_pool(name="state", bufs=1) as state_pool,
            tc.tile_pool(name="ps_s", bufs=3, space="PSUM") as ps_s,
        ):
            # Chunk 0 carries [e2t | W_0 init block]; later chunks carry the
            # per-step emission blocks.  Eager chunks go out immediately on
            # separate engine queues; rotating chunks are DMA'd lazily (the
            # pool WAR dep makes each wait until its buffer's previous chunk
            # has been consumed, so the transfers trail compute).
            dma_eng = [nc.sync, nc.gpsimd, nc.scalar] + [nc.sync] * NCHUNK
            gtiles = []
            gwidths = []
            for c in range(_NEAGER):
                c0 = PR + int(_BLK_OFF[_CHUNK_STEPS[c]]) if c > 0 else 0
                c1 = PR + int(_BLK_OFF[_CHUNK_STEPS[c + 1]])
                gt = ge_pool.tile([PR, c1 - c0], BF16, tag=f"g{c}")
                dma_eng[c].dma_start(gt[:], g_dram[:, c0:c1])
                gtiles.append(gt)
                gwidths.append(c1 - c0)

            e2t = gtiles[0][:, 0:PR]

            # Load blockdiag(E'^T, E'^T) into the PE array once; every step
            # matmul below reuses it (redundant auto-ldweights are stripped
            # after tile legalization below).
            nc.tensor.ldweights(e2t)

            # In-place state: step 1 is full width, so the state tile is
            # fully written by the first tensor_mul; the first matmul's
            # moving operand is the host-folded W_0 block of g directly.
            w_state = state_pool.tile([PR, WCOL], BF16, tag="w")

            chunk_of = np.searchsorted(_CHUNK_STEPS, np.arange(TSTEPS), "right") - 1
            next_chunk = _NEAGER
            for t in range(1, TSTEPS):
                c = int(chunk_of[t])
                # Issue each rotating chunk's DMA one chunk ahead of use;
                # the 2-buf pool WAR dep keeps transfers trailing compute.
                while next_chunk < NCHUNK and next_chunk <= c + 1:
                    cc = next_chunk
                    c0 = PR + int(_BLK_OFF[_CHUNK_STEPS[cc]])
                    c1 = PR + int(_BLK_OFF[_CHUNK_STEPS[cc + 1]])
                    gt = gr_pool.tile([PR, _ROT_W], BF16, tag="grot")
                    nc.sync.dma_start(gt[:, 0 : c1 - c0], g_dram[:, c0:c1])
                    gtiles.append(gt)
                    gwidths.append(c1 - c0)
                    next_chunk += 1
                n = int(_N_SCHED[t - 1])
                moving = (
                    gtiles[0][:, PR : PR + WCOL] if t == 1 else w_state[:, 0:n]
                )
                s_ps = ps_s.tile([PR, WCOL], F32, tag="s")
                nc.tensor.matmul(s_ps[:, 0:n], e2t, moving, start=True, stop=True)
                off = int(_BLK_OFF[t] - _BLK_OFF[_CHUNK_STEPS[c]])
                if c == 0:
                    off += PR
                nc.vector.tensor_mul(
                    w_state[:, 0:n], gtiles[c][:, off : off + n], s_ps[:, 0:n]
                )

            nc.sync.dma_start(wout_dram[:], w_state[:])

    # Tile legalization splits every bf16 matmult into LDWEIGHTS + MATMULT.
    # All those loads are of the SAME stationary tile, so keep only the
    # first (the explicit one above) and drop the rest.  The auto-inserted
    # loads carry no semaphore waits/updates (all sync lives on the
    # matmults), so removal is sync-neutral.
    kept_first = False
    for blk in nc.main_func.blocks:
        for i in list(blk.instructions):
            if isinstance(i, mybir.InstLdweights):
                if not kept_first:
                    kept_first = True
                elif i.sync_info is None:
                    blk.instructions.remove(i)

    nc.compile()
    return nc


_NC_CACHE = {}


def _get_nc():
    if "nc" not in _NC_CACHE:
        _NC_CACHE["nc"] = _build_nc()
    return _NC_CACHE["nc"]


def _host_norm(logit_b, len_b, T):
    """Exact float64 log-space forward for one sequence (fallback path)."""
    NEG_INF = -10000.0
    alpha = np.full(L, NEG_INF)
    alpha[START] = 0.0
    for t in range(len_b):
        mat = T + alpha[None, :]
        mx = mat.max(axis=1)
        alpha = logit_b[t] + np.log(np.exp(mat - mx[:, None]).sum(axis=1)) + mx
    v = alpha + T[STOP]
    mx = v.max()
    return np.log(np.exp(v - mx).sum()) + mx


def _prep_inputs(logits, lens, transitions):
    """Host-side preprocessing: exp + absorb-rewrite + deterministic
    per-(seq,step) scaling + length-sorted packing.  Stashes the float64
    log-scale accumulator, the column permutation, and any host-fallback
    results for _postprocess."""
    logits = np.asarray(logits, np.float32)
    lens = np.asarray(lens, np.int64)
    T = np.asarray(transitions, np.float64)

    E = np.exp(T)                      # [45,45] float64
    erow = E.mean(axis=1)              # mean_j E[i,j], [45]

    Eg = np.zeros((LD, LD), np.float64)
    Eg[:L, :L] = E
    Eg[DONE, :L] = E[STOP, :]
    Eg[DONE, DONE] = 1.0
    e2t = np.zeros((PR, PR), np.float64)
    e2t[:LD, :LD] = Eg.T
    e2t[LD:, LD:] = Eg.T

    G = np.exp(logits.astype(np.float64))          # [B,S,45]

    t_idx = np.arange(S)[None, :]                  # [1,S]
    active = t_idx < lens[:, None]                 # [B,S]

    # Fold step 0 and normalize it exactly: W0 = G0*E[:,START], scale 1/sum.
    W0 = G[:, 0, :] * E[:, START][None, :]         # [B,45]
    m0 = W0.sum(axis=1)                            # [B]
    G[:, 0, :] = W0 / m0[:, None]

    # Active steps t>=1: scale by 1/m_t, m_t = sum_i G_t[i]*erow[i].
    m = G @ erow                                   # [B,S]
    scale_mask = active & (t_idx > 0)
    np.divide(G, m[:, :, None], out=G, where=scale_mask[:, :, None])

    # log-scale accumulator: z[b] = log m0 + sum_{1<=t<len} log m_t.
    logm = np.where(scale_mask, np.log(m), 0.0)
    z = np.log(m0) + logm.sum(axis=1)

    # 46-label emissions: D gets 0 while active, onehot(D) from t>=len on.
    G46 = np.zeros((B, TSTEPS, LD), np.float64)
    G46[:, :S, :L] = np.where(active[:, :, None], G, 0.0)
    done_from = t_idx >= lens[:, None]             # includes absorb step
    G46[:, :S, DONE] = np.where(done_from, 1.0, 0.0)
    G46[:, S, DONE] = 1.0                          # appended step

    # Deal longest-first round-robin across the 16 (core, group) slots.
    order = np.argsort(-lens, kind="stable")
    slots = np.empty((NSLOTS, WCOL), np.int64)
    for r, b in enumerate(order):
        slots[r % NSLOTS][r // NSLOTS] = b
    # Host fallback for any sequence outliving its column's static lifetime.
    host_norms = {}
    logits64 = logits.astype(np.float64)
    for s in range(NSLOTS):
        for k in range(WCOL):
            b = slots[s][k]
            if lens[b] > _T_COL[k]:
                host_norms[int(b)] = _host_norm(logits64[b], int(lens[b]), T)

    _NC_CACHE["z"] = z
    _NC_CACHE["slots"] = slots
    _NC_CACHE["host_norms"] = host_norms

    g16 = G46.astype(NP_BF16)
    e2t16 = e2t.astype(NP_BF16)
    in_maps = []
    for c in range(NCORES):
        g_in = np.zeros((PR, PR + GCOLS), NP_BF16)
        g_in[:, :PR] = e2t16
        for g in range(NG):
            seqs = slots[c * NG + g]               # [WCOL] original indices
            rows = slice(g * LD, (g + 1) * LD)
            # Per-step blocks: step t occupies cols PR + [_BLK_OFF[t], +width).
            gc = g16[seqs]                         # [WCOL, TSTEPS, LD]
            for t in range(TSTEPS):
                w = int(_BLK_W[t])
                o = PR + int(_BLK_OFF[t])
                g_in[rows, o : o + w] = gc[:w, t, :].T
        in_maps.append({"g": g_in})
    return in_maps


def _postprocess(results, lens, transitions):
    z = _NC_CACHE["z"]
    slots = _NC_CACHE["slots"]
    host_norms = _NC_CACHE["host_norms"]
    norm = np.empty(B, np.float64)
    for c in range(NCORES):
        wout = np.asarray(results[c]["wout"]).astype(np.float64)  # [PR, WCOL]
        for g in range(NG):
            seqs = slots[c * NG + g]
            pdone = wout[g * LD + DONE, :]
            norm[seqs] = np.log(pdone) + z[seqs]
    for b, v in host_norms.items():
        norm[b] = v
    return norm.astype(np.float32)


def kernel(logits, lens, transitions):
    nc = _get_nc()
    in_maps = _prep_inputs(logits, lens, transitions)
    res = run_bass_kernel_spmd(nc, in_maps, list(range(NCORES)))
    return _postprocess(res.results, lens, transitions)


# revision 21
# speedup vs baseline: 1.0457x; 1.0164x over previous
"""CRF forward (log partition) on 8 NeuronCores, data-parallel over batch.

Math: the forward recurrence runs in probability space: with E = exp(T) and
G_t = exp(emissions_t), alpha_{t+1} = logit_t + LSE_j(T + alpha_t) becomes the
linear recurrence P_{t+1} = G_t o (E @ P_t).

All normalization is folded into the DATA on the host: each active step's
emission row is pre-scaled by 1/m_t[b] with m_t[b] = sum_i G[b,t,i]*rowmean(E)_i
(a deterministic per-sequence scalar), which keeps the state O(1) in bf16 range
without any data-dependent renorm on device.  The log-scales are accumulated in
float64 host-side and added back at the end.

Variable lengths via an extra DONE label D per group (46 labels on device):
E'[D,:45] = E[STOP,:], E'[D,D] = 1.0 (exact in bf16), column D otherwise 0.
Active steps emit 0 for D so P[D] stays exactly 0; the absorb step at t=len[b]
emits onehot(D), capturing LSE_j(T[STOP,j]+alpha_j) -- the final answer -- into
P[D]; later steps emit onehot(D) again, multiplying P[D] by exactly 1.0.

Shrinking-width steps: sequences are dealt longest-first round-robin across the
16 (core, group) slots, and within each slot sorted descending into columns, so
column k's sequence dies no later than a STATIC schedule width n_t allows.  The
state lives in ONE in-place tile; step t only updates columns [0, n_t), so dead
columns keep their DONE value frozen.  Any sequence too long for its column
(impossible under the static margin for uniform lengths, but checked) is
computed exactly on the host instead.

Per-core critical path per step: one bf16 matmul [92,92]x[92,n_t] with the
stationary blockdiag(E'^T,E'^T) kept loaded in the PE array (standalone
ldweights + non-self-loading matmuls), then one DVE tensor_mul.
"""

import numpy as np
import ml_dtypes

import concourse.bacc as bacc
import concourse.bass as bass
import concourse.mybir as mybir
import concourse.tile as tile
from concourse.bass_utils import run_bass_kernel_spmd

L = 45
START = 43
STOP = 44
LD = 46                    # labels + DONE landing pad
DONE = 45
B = 1024
S = 512
NCORES = 8
BPC = B // NCORES          # 128 sequences per core
NG = 2                     # groups per core
WCOL = BPC // NG           # 64 columns per group
PR = NG * LD               # 92 partition rows for packed state
TSTEPS = S + 1             # +1 appended absorb step
NSLOTS = NCORES * NG       # 16 (core, group) slots

F32 = mybir.dt.float32
BF16 = mybir.dt.bfloat16
NP_BF16 = ml_dtypes.bfloat16

# Static shrinking-width schedule: step t in [1, 512] updates columns [0, n_t).
# n_1 == WCOL always, so the in-place state is fully written by step 1.
_T_ARR = np.arange(1, TSTEPS)
_N_SCHED = np.minimum(
    WCOL, np.maximum(4, np.ceil(WCOL * (TSTEPS - _T_ARR) / TSTEPS).astype(int) + 2)
)
assert _N_SCHED[0] == WCOL
# Column lifetime: last step that still updates column k.
_T_COL = np.array(
    [int((np.where(_N_SCHED > k)[0] + 1).max()) for k in range(WCOL)], np.int64
)
# Per-step g block widths (block 0 is the full-width init state) and offsets.
_BLK_W = np.concatenate([[WCOL], _N_SCHED])          # [TSTEPS]
_BLK_OFF = np.concatenate([[0], np.cumsum(_BLK_W)])  # [TSTEPS+1]
GCOLS = int(_BLK_OFF[-1])
# Chunk boundaries (step indices): tiny leading chunks for a fast start; the
# later chunks rotate through a small pool so their DMAs trail compute.
_CHUNK_STEPS = [0, 1, 9, 41] + list(np.linspace(41, TSTEPS, 9).astype(int)[1:])
NCHUNK = len(_CHUNK_STEPS) - 1
_NEAGER = 3                      # chunks 0..2 load immediately, own tags
_ROT_W = int(
    max(
        _BLK_OFF[_CHUNK_STEPS[c + 1]] - _BLK_OFF[_CHUNK_STEPS[c]]
        for c in range(_NEAGER, NCHUNK)
    )
)


def _build_nc():
    # Bacc (not raw Bass): its legalization splits multi-sem waits into
    # standalone event-semaphore instructions, which walrus codegen requires.
    nc = bacc.Bacc("TRN2", target_bir_lowering=False, debug=False, num_devices=NCORES)
    # The stationary e2t matrix rides as the first PR columns of g, so one
    # DMA (and one semaphore) gates both the ldweights and the first matmul.
    g_dram = nc.dram_tensor("g", [PR, PR + GCOLS], BF16, kind="ExternalInput")
    wout_dram = nc.dram_tensor("wout", [PR, WCOL], BF16, kind="ExternalOutput")

    with tile.TileContext(nc) as tc:
        with (
            tc.tile_pool(name="geager", bufs=1) as ge_pool,
            tc.tile_pool(name="grot", bufs=2) as gr_pool,
            tc.tile_pool(name="state", bufs=1) as state_pool,
            tc.tile_pool(name="ps_s", bufs=3, space="PSUM") as ps_s,
        ):
            # Chunk 0 carries [e2t | W_0 init block]; later chunks carry the
            # per-step emission blocks.  Eager chunks go out immediately on
            # separate engine queues; rotating chunks are DMA'd lazily (the
            # pool WAR dep makes each wait until its buffer's previous chunk
            # has been consumed, so the transfers trail compute).
            dma_eng = [nc.sync] * (NCHUNK + 3)
            gtiles = []
            gwidths = []
            for c in range(_NEAGER):
                c0 = PR + int(_BLK_OFF[_CHUNK_STEPS[c]]) if c > 0 else 0
                c1 = PR + int(_BLK_OFF[_CHUNK_STEPS[c + 1]])
                gt = ge_pool.tile([PR, c1 - c0], BF16, tag=f"g{c}")
                dma_eng[c].dma_start(gt[:], g_dram[:, c0:c1])
                gtiles.append(gt)
                gwidths.append(c1 - c0)

            e2t = gtiles[0][:, 0:PR]

            # Load blockdiag(E'^T, E'^T) into the PE array once; every step
            # matmul below reuses it (redundant auto-ldweights are stripped
            # after tile legalization below).
            nc.tensor.ldweights(e2t)

            # In-place state: step 1 is full width, so the state tile is
            # fully written by the first tensor_mul; the first matmul's
            # moving operand is the host-folded W_0 block of g directly.
            w_state = state_pool.tile([PR, WCOL], BF16, tag="w")

            chunk_of = np.searchsorted(_CHUNK_STEPS, np.arange(TSTEPS), "right") - 1
            next_chunk = _NEAGER
            for t in range(1, TSTEPS):
                c = int(chunk_of[t])
                # Issue each rotating chunk's DMA one chunk ahead of use;
                # the 2-buf pool WAR dep keeps transfers trailing compute.
                while next_chunk < NCHUNK and next_chunk <= c + 1:
                    cc = next_chunk
                    c0 = PR + int(_BLK_OFF[_CHUNK_STEPS[cc]])
                    c1 = PR + int(_BLK_OFF[_CHUNK_STEPS[cc + 1]])
                    gt = gr_pool.tile([PR, _ROT_W], BF16, tag="grot")
                    nc.sync.dma_start(gt[:, 0 : c1 - c0], g_dram[:, c0:c1])
                    gtiles.append(gt)
                    gwidths.append(c1 - c0)
                    next_chunk += 1
                n = int(_N_SCHED[t - 1])
                moving = (
                    gtiles[0][:, PR : PR + WCOL] if t == 1 else w_state[:, 0:n]
                )
                s_ps = ps_s.tile([PR, WCOL], F32, tag="s")
                nc.tensor.matmul(s_ps[:, 0:n], e2t, moving, start=True, stop=True)
                off = int(_BLK_OFF[t] - _BLK_OFF[_CHUNK_STEPS[c]])
                if c == 0:
                    off += PR
                nc.vector.tensor_mul(
                    w_state[:, 0:n], gtiles[c][:, off : off + n], s_ps[:, 0:n]
                )

            nc.sync.dma_start(wout_dram[:], w_state[:])

    # Tile legalization splits every bf16 matmult into LDWEIGHTS + MATMULT.
    # All those loads are of the SAME stationary tile, so keep only the
    # first (the explicit one above) and drop the rest.  The auto-inserted
    # loads carry no semaphore waits/updates (all sync lives on the
    # matmults), so removal is sync-neutral.
    kept_first = False
    for blk in nc.main_func.blocks:
        for i in list(blk.instructions):
            if isinstance(i, mybir.InstLdweights):
                if not kept_first:
                    kept_first = True
                elif i.sync_info is None:
                    blk.instructions.remove(i)

    nc.compile()
    return nc


_NC_CACHE = {}


def _get_nc():
    if "nc" not in _NC_CACHE:
        _NC_CACHE["nc"] = _build_nc()
    return _NC_CACHE["nc"]


def _host_norm(logit_b, len_b, T):
    """Exact float64 log-space forward for one sequence (fallback path)."""
    NEG_INF = -10000.0
    alpha = np.full(L, NEG_INF)
    alpha[START] = 0.0
    for t in range(len_b):
        mat = T + alpha[None, :]
        mx = mat.max(axis=1)
        alpha = logit_b[t] + np.log(np.exp(mat - mx[:, None]).sum(axis=1)) + mx
    v = alpha + T[STOP]
    mx = v.max()
    return np.log(np.exp(v - mx).sum()) + mx


def _prep_inputs(logits, lens, transitions):
    """Host-side preprocessing: exp + absorb-rewrite + deterministic
    per-(seq,step) scaling + length-sorted packing.  Stashes the float64
    log-scale accumulator, the column permutation, and any host-fallback
    results for _postprocess."""
    logits = np.asarray(logits, np.float32)
    lens = np.asarray(lens, np.int64)
    T = np.asarray(transitions, np.float64)

    E = np.exp(T)                      # [45,45] float64
    erow = E.mean(axis=1)              # mean_j E[i,j], [45]

    Eg = np.zeros((LD, LD), np.float64)
    Eg[:L, :L] = E
    Eg[DONE, :L] = E[STOP, :]
    Eg[DONE, DONE] = 1.0
    e2t = np.zeros((PR, PR), np.float64)
    e2t[:LD, :LD] = Eg.T
    e2t[LD:, LD:] = Eg.T

    G = np.exp(logits.astype(np.float64))          # [B,S,45]

    t_idx = np.arange(S)[None, :]                  # [1,S]
    active = t_idx < lens[:, None]                 # [B,S]

    # Fold step 0 and normalize it exactly: W0 = G0*E[:,START], scale 1/sum.
    W0 = G[:, 0, :] * E[:, START][None, :]         # [B,45]
    m0 = W0.sum(axis=1)                            # [B]
    G[:, 0, :] = W0 / m0[:, None]

    # Active steps t>=1: scale by 1/m_t, m_t = sum_i G_t[i]*erow[i].
    m = G @ erow                                   # [B,S]
    scale_mask = active & (t_idx > 0)
    np.divide(G, m[:, :, None], out=G, where=scale_mask[:, :, None])

    # log-scale accumulator: z[b] = log m0 + sum_{1<=t<len} log m_t.
    logm = np.where(scale_mask, np.log(m), 0.0)
    z = np.log(m0) + logm.sum(axis=1)

    # 46-label emissions: D gets 0 while active, onehot(D) from t>=len on.
    G46 = np.zeros((B, TSTEPS, LD), np.float64)
    G46[:, :S, :L] = np.where(active[:, :, None], G, 0.0)
    done_from = t_idx >= lens[:, None]             # includes absorb step
    G46[:, :S, DONE] = np.where(done_from, 1.0, 0.0)
    G46[:, S, DONE] = 1.0                          # appended step

    # Deal longest-first round-robin across the 16 (core, group) slots.
    order = np.argsort(-lens, kind="stable")
    slots = np.empty((NSLOTS, WCOL), np.int64)
    for r, b in enumerate(order):
        slots[r % NSLOTS][r // NSLOTS] = b
    # Host fallback for any sequence outliving its column's static lifetime.
    host_norms = {}
    logits64 = logits.astype(np.float64)
    for s in range(NSLOTS):
        for k in range(WCOL):
            b = slots[s][k]
            if lens[b] > _T_COL[k]:
                host_norms[int(b)] = _host_norm(logits64[b], int(lens[b]), T)

    _NC_CACHE["z"] = z
    _NC_CACHE["slots"] = slots
    _NC_CACHE["host_norms"] = host_norms

    g16 = G46.astype(NP_BF16)
    e2t16 = e2t.astype(NP_BF16)
    in_maps = []
    for c in range(NCORES):
        g_in = np.zeros((PR, PR + GCOLS), NP_BF16)
        g_in[:, :PR] = e2t16
        for g in range(NG):
            seqs = slots[c * NG + g]               # [WCOL] original indices
            rows = slice(g * LD, (g + 1) * LD)
            # Per-step blocks: step t occupies cols PR + [_BLK_OFF[t], +width).
            gc = g16[seqs]                         # [WCOL, TSTEPS, LD]
            for t in range(TSTEPS):
                w = int(_BLK_W[t])
                o = PR + int(_BLK_OFF[t])
                g_in[rows, o : o + w] = gc[:w, t, :].T
        in_maps.append({"g": g_in})
    return in_maps


def _postprocess(results, lens, transitions):
    z = _NC_CACHE["z"]
    slots = _NC_CACHE["slots"]
    host_norms = _NC_CACHE["host_norms"]
    norm = np.empty(B, np.float64)
    for c in range(NCORES):
        wout = np.asarray(results[c]["wout"]).astype(np.float64)  # [PR, WCOL]
        for g in range(NG):
            seqs = slots[c * NG + g]
            pdone = wout[g * LD + DONE, :]
            norm[seqs] = np.log(pdone) + z[seqs]
    for b, v in host_norms.items():
        norm[b] = v
    return norm.astype(np.float32)


def kernel(logits, lens, transitions):
    nc = _get_nc()
    in_maps = _prep_inputs(logits, lens, transitions)
    res = run_bass_kernel_spmd(nc, in_maps, list(range(NCORES)))
    return _postprocess(res.results, lens, transitions)


# revision 22
# speedup vs baseline: 1.0487x; 1.0029x over previous
"""CRF forward (log partition) on 8 NeuronCores, data-parallel over batch.

Math: the forward recurrence runs in probability space: with E = exp(T) and
G_t = exp(emissions_t), alpha_{t+1} = logit_t + LSE_j(T + alpha_t) becomes the
linear recurrence P_{t+1} = G_t o (E @ P_t).

All normalization is folded into the DATA on the host: each active step's
emission row is pre-scaled by 1/m_t[b] with m_t[b] = sum_i G[b,t,i]*rowmean(E)_i
(a deterministic per-sequence scalar), which keeps the state O(1) in bf16 range
without any data-dependent renorm on device.  The log-scales are accumulated in
float64 host-side and added back at the end.

Variable lengths via an extra DONE label D per group (46 labels on device):
E'[D,:45] = E[STOP,:], E'[D,D] = 1.0 (exact in bf16), column D otherwise 0.
Active steps emit 0 for D so P[D] stays exactly 0; the absorb step at t=len[b]
emits onehot(D), capturing LSE_j(T[STOP,j]+alpha_j) -- the final answer -- into
P[D]; later steps emit onehot(D) again, multiplying P[D] by exactly 1.0.

Shrinking-width steps: sequences are dealt longest-first round-robin across the
16 (core, group) slots, and within each slot sorted descending into columns, so
column k's sequence dies no later than a STATIC schedule width n_t allows.  The
state lives in ONE in-place tile; step t only updates columns [0, n_t), so dead
columns keep their DONE value frozen.  Any sequence too long for its column
(impossible under the static margin for uniform lengths, but checked) is
computed exactly on the host instead.

Per-core critical path per step: one bf16 matmul [92,92]x[92,n_t] with the
stationary blockdiag(E'^T,E'^T) kept loaded in the PE array (standalone
ldweights + non-self-loading matmuls), then one DVE tensor_mul.
"""

import numpy as np
import ml_dtypes

import concourse.bacc as bacc
import concourse.bass as bass
import concourse.mybir as mybir
import concourse.tile as tile
from concourse.bass_utils import run_bass_kernel_spmd

L = 45
START = 43
STOP = 44
LD = 46                    # labels + DONE landing pad
DONE = 45
B = 1024
S = 512
NCORES = 8
BPC = B // NCORES          # 128 sequences per core
NG = 2                     # groups per core
WCOL = BPC // NG           # 64 columns per group
PR = NG * LD               # 92 partition rows for packed state
TSTEPS = S + 1             # +1 appended absorb step
NSLOTS = NCORES * NG       # 16 (core, group) slots

F32 = mybir.dt.float32
BF16 = mybir.dt.bfloat16
NP_BF16 = ml_dtypes.bfloat16

# Static shrinking-width schedule: step t in [1, 512] updates columns [0, n_t).
# n_1 == WCOL always, so the in-place state is fully written by step 1.
_T_ARR = np.arange(1, TSTEPS)
_N_SCHED = np.minimum(
    WCOL, np.maximum(4, np.ceil(WCOL * (TSTEPS - _T_ARR) / TSTEPS).astype(int) + 2)
)
assert _N_SCHED[0] == WCOL
# Column lifetime: last step that still updates column k.
_T_COL = np.array(
    [int((np.where(_N_SCHED > k)[0] + 1).max()) for k in range(WCOL)], np.int64
)
# Per-step g block widths (block 0 is the full-width init state) and offsets.
_BLK_W = np.concatenate([[WCOL], _N_SCHED])          # [TSTEPS]
_BLK_OFF = np.concatenate([[0], np.cumsum(_BLK_W)])  # [TSTEPS+1]
GCOLS = int(_BLK_OFF[-1])
# Chunk boundaries (step indices): tiny leading chunks for a fast start; the
# later chunks rotate through a small pool so their DMAs trail compute.
_CHUNK_STEPS = [0, 9, 41] + list(np.linspace(41, TSTEPS, 9).astype(int)[1:])
NCHUNK = len(_CHUNK_STEPS) - 1
_NEAGER = 2                      # chunks 0..1 load immediately, own tags
_ROT_W = int(
    max(
        _BLK_OFF[_CHUNK_STEPS[c + 1]] - _BLK_OFF[_CHUNK_STEPS[c]]
        for c in range(_NEAGER, NCHUNK)
    )
)


def _build_nc():
    # Bacc (not raw Bass): its legalization splits multi-sem waits into
    # standalone event-semaphore instructions, which walrus codegen requires.
    nc = bacc.Bacc("TRN2", target_bir_lowering=False, debug=False, num_devices=NCORES)
    # The stationary e2t matrix rides as the first PR columns of g, so one
    # DMA (and one semaphore) gates both the ldweights and the first matmul.
    g_dram = nc.dram_tensor("g", [PR, PR + GCOLS], BF16, kind="ExternalInput")
    wout_dram = nc.dram_tensor("wout", [PR, WCOL], BF16, kind="ExternalOutput")

    with tile.TileContext(nc) as tc:
        with (
            tc.tile_pool(name="geager", bufs=1) as ge_pool,
            tc.tile_pool(name="grot", bufs=2) as gr_pool,
            tc.tile_pool(name="state", bufs=1) as state_pool,
            tc.tile_pool(name="ps_s", bufs=3, space="PSUM") as ps_s,
        ):
            # Chunk 0 carries [e2t | W_0 init block]; later chunks carry the
            # per-step emission blocks.  Eager chunks go out immediately on
            # separate engine queues; rotating chunks are DMA'd lazily (the
            # pool WAR dep makes each wait until its buffer's previous chunk
            # has been consumed, so the transfers trail compute).
            dma_eng = [nc.sync] * (NCHUNK + 3)
            gtiles = []
            gwidths = []
            for c in range(_NEAGER):
                c0 = PR + int(_BLK_OFF[_CHUNK_STEPS[c]]) if c > 0 else 0
                c1 = PR + int(_BLK_OFF[_CHUNK_STEPS[c + 1]])
                gt = ge_pool.tile([PR, c1 - c0], BF16, tag=f"g{c}")
                dma_eng[c].dma_start(gt[:], g_dram[:, c0:c1])
                gtiles.append(gt)
                gwidths.append(c1 - c0)

            e2t = gtiles[0][:, 0:PR]

            # Load blockdiag(E'^T, E'^T) into the PE array once; every step
            # matmul below reuses it (redundant auto-ldweights are stripped
            # after tile legalization below).
            nc.tensor.ldweights(e2t)

            # In-place state: step 1 is full width, so the state tile is
            # fully written by the first tensor_mul; the first matmul's
            # moving operand is the host-folded W_0 block of g directly.
            w_state = state_pool.tile([PR, WCOL], BF16, tag="w")

            chunk_of = np.searchsorted(_CHUNK_STEPS, np.arange(TSTEPS), "right") - 1
            next_chunk = _NEAGER
            for t in range(1, TSTEPS):
                c = int(chunk_of[t])
                # Issue each rotating chunk's DMA one chunk ahead of use;
                # the 2-buf pool WAR dep keeps transfers trailing compute.
                while next_chunk < NCHUNK and next_chunk <= c + 1:
                    cc = next_chunk
                    c0 = PR + int(_BLK_OFF[_CHUNK_STEPS[cc]])
                    c1 = PR + int(_BLK_OFF[_CHUNK_STEPS[cc + 1]])
                    gt = gr_pool.tile([PR, _ROT_W], BF16, tag="grot")
                    nc.sync.dma_start(gt[:, 0 : c1 - c0], g_dram[:, c0:c1])
                    gtiles.append(gt)
                    gwidths.append(c1 - c0)
                    next_chunk += 1
                n = int(_N_SCHED[t - 1])
                moving = (
                    gtiles[0][:, PR : PR + WCOL] if t == 1 else w_state[:, 0:n]
                )
                s_ps = ps_s.tile([PR, WCOL], F32, tag="s")
                nc.tensor.matmul(s_ps[:, 0:n], e2t, moving, start=True, stop=True)
                off = int(_BLK_OFF[t] - _BLK_OFF[_CHUNK_STEPS[c]])
                if c == 0:
                    off += PR
                nc.vector.tensor_mul(
                    w_state[:, 0:n], gtiles[c][:, off : off + n], s_ps[:, 0:n]
                )

            nc.sync.dma_start(wout_dram[:], w_state[:])

    # Tile legalization splits every bf16 matmult into LDWEIGHTS + MATMULT.
    # All those loads are of the SAME stationary tile, so keep only the
    # first (the explicit one above) and drop the rest.  The auto-inserted
    # loads carry no semaphore waits/updates (all sync lives on the
    # matmults), so removal is sync-neutral.
    kept_first = False
    for blk in nc.main_func.blocks:
        for i in list(blk.instructions):
            if isinstance(i, mybir.InstLdweights):
                if not kept_first:
                    kept_first = True
                elif i.sync_info is None:
                    blk.instructions.remove(i)

    nc.compile()
    return nc


_NC_CACHE = {}


def _get_nc():
    if "nc" not in _NC_CACHE:
        _NC_CACHE["nc"] = _build_nc()
    return _NC_CACHE["nc"]


def _host_norm(logit_b, len_b, T):
    """Exact float64 log-space forward for one sequence (fallback path)."""
    NEG_INF = -10000.0
    alpha = np.full(L, NEG_INF)
    alpha[START] = 0.0
    for t in range(len_b):
        mat = T + alpha[None, :]
        mx = mat.max(axis=1)
        alpha = logit_b[t] + np.log(np.exp(mat - mx[:, None]).sum(axis=1)) + mx
    v = alpha + T[STOP]
    mx = v.max()
    return np.log(np.exp(v - mx).sum()) + mx


def _prep_inputs(logits, lens, transitions):
    """Host-side preprocessing: exp + absorb-rewrite + deterministic
    per-(seq,step) scaling + length-sorted packing.  Stashes the float64
    log-scale accumulator, the column permutation, and any host-fallback
    results for _postprocess."""
    logits = np.asarray(logits, np.float32)
    lens = np.asarray(lens, np.int64)
    T = np.asarray(transitions, np.float64)

    E = np.exp(T)                      # [45,45] float64
    erow = E.mean(axis=1)              # mean_j E[i,j], [45]

    Eg = np.zeros((LD, LD), np.float64)
    Eg[:L, :L] = E
    Eg[DONE, :L] = E[STOP, :]
    Eg[DONE, DONE] = 1.0
    e2t = np.zeros((PR, PR), np.float64)
    e2t[:LD, :LD] = Eg.T
    e2t[LD:, LD:] = Eg.T

    G = np.exp(logits.astype(np.float64))          # [B,S,45]

    t_idx = np.arange(S)[None, :]                  # [1,S]
    active = t_idx < lens[:, None]                 # [B,S]

    # Fold step 0 and normalize it exactly: W0 = G0*E[:,START], scale 1/sum.
    W0 = G[:, 0, :] * E[:, START][None, :]         # [B,45]
    m0 = W0.sum(axis=1)                            # [B]
    G[:, 0, :] = W0 / m0[:, None]

    # Active steps t>=1: scale by 1/m_t, m_t = sum_i G_t[i]*erow[i].
    m = G @ erow                                   # [B,S]
    scale_mask = active & (t_idx > 0)
    np.divide(G, m[:, :, None], out=G, where=scale_mask[:, :, None])

    # log-scale accumulator: z[b] = log m0 + sum_{1<=t<len} log m_t.
    logm = np.where(scale_mask, np.log(m), 0.0)
    z = np.log(m0) + logm.sum(axis=1)

    # 46-label emissions: D gets 0 while active, onehot(D) from t>=len on.
    G46 = np.zeros((B, TSTEPS, LD), np.float64)
    G46[:, :S, :L] = np.where(active[:, :, None], G, 0.0)
    done_from = t_idx >= lens[:, None]             # includes absorb step
    G46[:, :S, DONE] = np.where(done_from, 1.0, 0.0)
    G46[:, S, DONE] = 1.0                          # appended step

    # Deal longest-first round-robin across the 16 (core, group) slots.
    order = np.argsort(-lens, kind="stable")
    slots = np.empty((NSLOTS, WCOL), np.int64)
    for r, b in enumerate(order):
        slots[r % NSLOTS][r // NSLOTS] = b
    # Host fallback for any sequence outliving its column's static lifetime.
    host_norms = {}
    logits64 = logits.astype(np.float64)
    for s in range(NSLOTS):
        for k in range(WCOL):
            b = slots[s][k]
            if lens[b] > _T_COL[k]:
                host_norms[int(b)] = _host_norm(logits64[b], int(lens[b]), T)

    _NC_CACHE["z"] = z
    _NC_CACHE["slots"] = slots
    _NC_CACHE["host_norms"] = host_norms

    g16 = G46.astype(NP_BF16)
    e2t16 = e2t.astype(NP_BF16)
    in_maps = []
    for c in range(NCORES):
        g_in = np.zeros((PR, PR + GCOLS), NP_BF16)
        g_in[:, :PR] = e2t16
        for g in range(NG):
            seqs = slots[c * NG + g]               # [WCOL] original indices
            rows = slice(g * LD, (g + 1) * LD)
            # Per-step blocks: step t occupies cols PR + [_BLK_OFF[t], +width).
            gc = g16[seqs]                         # [WCOL, TSTEPS, LD]
            for t in range(TSTEPS):
                w = int(_BLK_W[t])
                o = PR + int(_BLK_OFF[t])
                g_in[rows, o : o + w] = gc[:w, t, :].T
        in_maps.append({"g": g_in})
    return in_maps


def _postprocess(results, lens, transitions):
    z = _NC_CACHE["z"]
    slots = _NC_CACHE["slots"]
    host_norms = _NC_CACHE["host_norms"]
    norm = np.empty(B, np.float64)
    for c in range(NCORES):
        wout = np.asarray(results[c]["wout"]).astype(np.float64)  # [PR, WCOL]
        for g in range(NG):
            seqs = slots[c * NG + g]
            pdone = wout[g * LD + DONE, :]
            norm[seqs] = np.log(pdone) + z[seqs]
    for b, v in host_norms.items():
        norm[b] = v
    return norm.astype(np.float32)


def kernel(logits, lens, transitions):
    nc = _get_nc()
    in_maps = _prep_inputs(logits, lens, transitions)
    res = run_bass_kernel_spmd(nc, in_maps, list(range(NCORES)))
    return _postprocess(res.results, lens, transitions)


# revision 23
# speedup vs baseline: 1.0533x; 1.0044x over previous
"""CRF forward (log partition) on 8 NeuronCores, data-parallel over batch.

Math: the forward recurrence runs in probability space: with E = exp(T) and
G_t = exp(emissions_t), alpha_{t+1} = logit_t + LSE_j(T + alpha_t) becomes the
linear recurrence P_{t+1} = G_t o (E @ P_t).

All normalization is folded into the DATA on the host: each active step's
emission row is pre-scaled by 1/m_t[b] with m_t[b] = sum_i G[b,t,i]*rowmean(E)_i
(a deterministic per-sequence scalar), which keeps the state O(1) in bf16 range
without any data-dependent renorm on device.  The log-scales are accumulated in
float64 host-side and added back at the end.

Variable lengths via an extra DONE label D per group (46 labels on device):
E'[D,:45] = E[STOP,:], E'[D,D] = 1.0 (exact in bf16), column D otherwise 0.
Active steps emit 0 for D so P[D] stays exactly 0; the absorb step at t=len[b]
emits onehot(D), capturing LSE_j(T[STOP,j]+alpha_j) -- the final answer -- into
P[D]; later steps emit onehot(D) again, multiplying P[D] by exactly 1.0.

Shrinking-width steps: sequences are dealt longest-first round-robin across the
16 (core, group) slots and sorted descending into columns, and the per-step
instruction width n_t is derived from the actual length distribution (exact,
so no sequence outlives its column; a host fallback still guards arbitrary
inputs).  The state lives in ONE in-place tile; step t only updates columns
[0, n_t), so dead columns keep their DONE value frozen.

Per-core critical path per step: one bf16 matmul [92,92]x[92,n_t] with the
stationary blockdiag(E'^T,E'^T) kept loaded in the PE array (standalone
ldweights + non-self-loading matmuls), then one DVE tensor_mul.
"""

import numpy as np
import ml_dtypes

import concourse.bacc as bacc
import concourse.bass as bass
import concourse.mybir as mybir
import concourse.tile as tile
from concourse.bass_utils import run_bass_kernel_spmd

L = 45
START = 43
STOP = 44
LD = 46                    # labels + DONE landing pad
DONE = 45
B = 1024
S = 512
NCORES = 8
BPC = B // NCORES          # 128 sequences per core
NG = 2                     # groups per core
WCOL = BPC // NG           # 64 columns per group
PR = NG * LD               # 92 partition rows for packed state
TSTEPS = S + 1             # +1 appended absorb step
NSLOTS = NCORES * NG       # 16 (core, group) slots

F32 = mybir.dt.float32
BF16 = mybir.dt.bfloat16
NP_BF16 = ml_dtypes.bfloat16


class _Plan:
    """Width schedule + g layout + chunking, derived from the lengths."""

    def __init__(self, n_sched):
        n_sched = np.asarray(n_sched, np.int64)
        assert n_sched.shape == (S,) and n_sched[0] == WCOL
        self.n_sched = n_sched
        # Column lifetime: last step that still updates column k.
        self.t_col = np.array(
            [
                int((np.where(n_sched > k)[0] + 1).max()) if (n_sched > k).any() else 0
                for k in range(WCOL)
            ],
            np.int64,
        )
        # Per-step g block widths (block 0 is the full-width init state).
        self.blk_w = np.concatenate([[WCOL], n_sched])          # [TSTEPS]
        self.blk_off = np.concatenate([[0], np.cumsum(self.blk_w)])
        self.gcols = int(self.blk_off[-1])
        # Chunk boundaries (step indices): small leading chunks start the
        # pipeline fast; later chunks rotate through a 2-buffer pool with
        # lazily issued DMAs so the transfers trail compute.
        self.chunk_steps = [0, 9, 41] + list(np.linspace(41, TSTEPS, 9).astype(int)[1:])
        self.nchunk = len(self.chunk_steps) - 1
        self.neager = 2
        self.rot_w = int(
            max(
                self.blk_off[self.chunk_steps[c + 1]] - self.blk_off[self.chunk_steps[c]]
                for c in range(self.neager, self.nchunk)
            )
        )
        self.key = n_sched.tobytes()


def _build_nc(plan):
    # Bacc (not raw Bass): its legalization splits multi-sem waits into
    # standalone event-semaphore instructions, which walrus codegen requires.
    nc = bacc.Bacc("TRN2", target_bir_lowering=False, debug=False, num_devices=NCORES)
    # The stationary e2t matrix rides as the first PR columns of g, so one
    # DMA (and one semaphore) gates both the ldweights and the first matmul.
    g_dram = nc.dram_tensor("g", [PR, PR + plan.gcols], BF16, kind="ExternalInput")
    wout_dram = nc.dram_tensor("wout", [PR, WCOL], BF16, kind="ExternalOutput")

    cs = plan.chunk_steps
    with tile.TileContext(nc) as tc:
        with (
            tc.tile_pool(name="geager", bufs=1) as ge_pool,
            tc.tile_pool(name="grot", bufs=2) as gr_pool,
            tc.tile_pool(name="state", bufs=1) as state_pool,
            tc.tile_pool(name="ps_s", bufs=3, space="PSUM") as ps_s,
        ):
            # Chunk 0 carries [e2t | W_0 init | steps 1..8]; chunk 1 the next
            # steps.  Both load immediately on the sync DMA queue.
            gtiles = []
            for c in range(plan.neager):
                c0 = PR + int(plan.blk_off[cs[c]]) if c > 0 else 0
                c1 = PR + int(plan.blk_off[cs[c + 1]])
                gt = ge_pool.tile([PR, c1 - c0], BF16, tag=f"g{c}")
                nc.sync.dma_start(gt[:], g_dram[:, c0:c1])
                gtiles.append(gt)

            e2t = gtiles[0][:, 0:PR]

            # Load blockdiag(E'^T, E'^T) into the PE array once; every step
            # matmul below reuses it (redundant auto-ldweights are stripped
            # after tile legalization below).
            nc.tensor.ldweights(e2t)

            # In-place state: step 1 is full width, so the state tile is
            # fully written by the first tensor_mul; the first matmul's
            # moving operand is the host-folded W_0 block of g directly.
            w_state = state_pool.tile([PR, WCOL], BF16, tag="w")

            chunk_of = np.searchsorted(cs, np.arange(TSTEPS), "right") - 1
            next_chunk = plan.neager
            for t in range(1, TSTEPS):
                # Issue each rotating chunk's DMA ~16 steps before its first
                # use; the 2-buf pool WAR dep keeps transfers trailing compute.
                while next_chunk < plan.nchunk and t >= cs[next_chunk] - 16:
                    cc = next_chunk
                    c0 = PR + int(plan.blk_off[cs[cc]])
                    c1 = PR + int(plan.blk_off[cs[cc + 1]])
                    gt = gr_pool.tile([PR, plan.rot_w], BF16, tag="grot")
                    nc.sync.dma_start(gt[:, 0 : c1 - c0], g_dram[:, c0:c1])
                    gtiles.append(gt)
                    next_chunk += 1
                n = int(plan.n_sched[t - 1])
                c = int(chunk_of[t])
                moving = (
                    gtiles[0][:, PR : PR + WCOL] if t == 1 else w_state[:, 0:n]
                )
                s_ps = ps_s.tile([PR, WCOL], F32, tag="s")
                nc.tensor.matmul(s_ps[:, 0:n], e2t, moving, start=True, stop=True)
                off = int(plan.blk_off[t] - plan.blk_off[cs[c]])
                if c == 0:
                    off += PR
                nc.vector.tensor_mul(
                    w_state[:, 0:n], gtiles[c][:, off : off + n], s_ps[:, 0:n]
                )

            nc.sync.dma_start(wout_dram[:], w_state[:])

    # Tile legalization splits every bf16 matmult into LDWEIGHTS + MATMULT.
    # All those loads are of the SAME stationary tile, so keep only the
    # first (the explicit one above) and drop the rest.  The auto-inserted
    # loads carry no semaphore waits/updates (all sync lives on the
    # matmults), so removal is sync-neutral.
    kept_first = False
    for blk in nc.main_func.blocks:
        for i in list(blk.instructions):
            if isinstance(i, mybir.InstLdweights):
                if not kept_first:
                    kept_first = True
                elif i.sync_info is None:
                    blk.instructions.remove(i)

    nc.compile()
    return nc


_NC_CACHE = {}


def _get_nc():
    """Return the nc built for the most recent _prep_inputs call."""
    return _NC_CACHE["nc"]


def _host_norm(logit_b, len_b, T):
    """Exact float64 log-space forward for one sequence (fallback path)."""
    NEG_INF = -10000.0
    alpha = np.full(L, NEG_INF)
    alpha[START] = 0.0
    for t in range(len_b):
        mat = T + alpha[None, :]
        mx = mat.max(axis=1)
        alpha = logit_b[t] + np.log(np.exp(mat - mx[:, None]).sum(axis=1)) + mx
    v = alpha + T[STOP]
    mx = v.max()
    return np.log(np.exp(v - mx).sum()) + mx


def _prep_inputs(logits, lens, transitions):
    """Host-side preprocessing: exp + absorb-rewrite + deterministic
    per-(seq,step) scaling + length-sorted packing.  Derives the width
    schedule from the actual lengths, builds (or reuses) the matching nc,
    and stashes everything _postprocess needs."""
    logits = np.asarray(logits, np.float32)
    lens = np.asarray(lens, np.int64)
    T = np.asarray(transitions, np.float64)

    E = np.exp(T)                      # [45,45] float64
    erow = E.mean(axis=1)              # mean_j E[i,j], [45]

    Eg = np.zeros((LD, LD), np.float64)
    Eg[:L, :L] = E
    Eg[DONE, :L] = E[STOP, :]
    Eg[DONE, DONE] = 1.0
    e2t = np.zeros((PR, PR), np.float64)
    e2t[:LD, :LD] = Eg.T
    e2t[LD:, LD:] = Eg.T

    G = np.exp(logits.astype(np.float64))          # [B,S,45]

    t_idx = np.arange(S)[None, :]                  # [1,S]
    active = t_idx < lens[:, None]                 # [B,S]

    # Fold step 0 and normalize it exactly: W0 = G0*E[:,START], scale 1/sum.
    W0 = G[:, 0, :] * E[:, START][None, :]         # [B,45]
    m0 = W0.sum(axis=1)                            # [B]
    G[:, 0, :] = W0 / m0[:, None]

    # Active steps t>=1: scale by 1/m_t, m_t = sum_i G_t[i]*erow[i].
    m = G @ erow                                   # [B,S]
    scale_mask = active & (t_idx > 0)
    np.divide(G, m[:, :, None], out=G, where=scale_mask[:, :, None])

    # log-scale accumulator: z[b] = log m0 + sum_{1<=t<len} log m_t.
    logm = np.where(scale_mask, np.log(m), 0.0)
    z = np.log(m0) + logm.sum(axis=1)

    # 46-label emissions: D gets 0 while active, onehot(D) from t>=len on.
    G46 = np.zeros((B, TSTEPS, LD), np.float64)
    G46[:, :S, :L] = np.where(active[:, :, None], G, 0.0)
    done_from = t_idx >= lens[:, None]             # includes absorb step
    G46[:, :S, DONE] = np.where(done_from, 1.0, 0.0)
    G46[:, S, DONE] = 1.0                          # appended step

    # Deal longest-first round-robin across the 16 (core, group) slots.
    order = np.argsort(-lens, kind="stable")
    slots = np.empty((NSLOTS, WCOL), np.int64)
    for r, b in enumerate(order):
        slots[r % NSLOTS][r // NSLOTS] = b
    slot_lens = lens[slots]                        # [NSLOTS, WCOL] descending

    # Exact width schedule: step t updates max-over-slots active columns
    # (clamped so step 1 is full width and the schedule is positive).
    steps = np.arange(1, TSTEPS)                   # [S]
    n_sched = (slot_lens[:, :, None] >= steps[None, None, :]).sum(axis=1).max(axis=0)
    n_sched = np.maximum(n_sched, 1)
    n_sched[0] = WCOL
    n_sched = np.maximum.accumulate(n_sched[::-1])[::-1]  # non-increasing
    plan = _Plan(n_sched)

    if _NC_CACHE.get("key") != plan.key:
        _NC_CACHE["nc"] = _build_nc(plan)
        _NC_CACHE["key"] = plan.key

    # Host fallback for any sequence outliving its column (none when the
    # schedule is derived from these lens, but guards future reuse).
    host_norms = {}
    logits64 = logits.astype(np.float64)
    for s in range(NSLOTS):
        for k in range(WCOL):
            b = slots[s][k]
            if lens[b] > plan.t_col[k]:
                host_norms[int(b)] = _host_norm(logits64[b], int(lens[b]), T)

    _NC_CACHE["plan"] = plan
    _NC_CACHE["z"] = z
    _NC_CACHE["slots"] = slots
    _NC_CACHE["host_norms"] = host_norms

    g16 = G46.astype(NP_BF16)
    e2t16 = e2t.astype(NP_BF16)
    in_maps = []
    for c in range(NCORES):
        g_in = np.zeros((PR, PR + plan.gcols), NP_BF16)
        g_in[:, :PR] = e2t16
        for g in range(NG):
            seqs = slots[c * NG + g]               # [WCOL] original indices
            rows = slice(g * LD, (g + 1) * LD)
            # Per-step blocks: step t occupies cols PR + [blk_off[t], +width).
            gc = g16[seqs]                         # [WCOL, TSTEPS, LD]
            for t in range(TSTEPS):
                w = int(plan.blk_w[t])
                o = PR + int(plan.blk_off[t])
                g_in[rows, o : o + w] = gc[:w, t, :].T
        in_maps.append({"g": g_in})
    return in_maps


def _postprocess(results, lens, transitions):
    z = _NC_CACHE["z"]
    slots = _NC_CACHE["slots"]
    host_norms = _NC_CACHE["host_norms"]
    norm = np.empty(B, np.float64)
    for c in range(NCORES):
        wout = np.asarray(results[c]["wout"]).astype(np.float64)  # [PR, WCOL]
        for g in range(NG):
            seqs = slots[c * NG + g]
            pdone = wout[g * LD + DONE, :]
            norm[seqs] = np.log(pdone) + z[seqs]
    for b, v in host_norms.items():
        norm[b] = v
    return norm.astype(np.float32)


def kernel(logits, lens, transitions):
    in_maps = _prep_inputs(logits, lens, transitions)
    nc = _get_nc()
    res = run_bass_kernel_spmd(nc, in_maps, list(range(NCORES)))
    return _postprocess(res.results, lens, transitions)


# revision 24
# speedup vs baseline: 1.0591x; 1.0055x over previous
"""CRF forward (log partition) on 8 NeuronCores, data-parallel over batch.

Math: the forward recurrence runs in probability space: with E = exp(T) and
G_t = exp(emissions_t), alpha_{t+1} = logit_t + LSE_j(T + alpha_t) becomes the
linear recurrence P_{t+1} = G_t o (E @ P_t).

All normalization is folded into the DATA on the host: each active step's
emission row is pre-scaled by 1/m_t[b] with m_t[b] = sum_i G[b,t,i]*rowmean(E)_i
(a deterministic per-sequence scalar), which keeps the state O(1) in bf16 range
without any data-dependent renorm on device.  The log-scales are accumulated in
float64 host-side and added back at the end.

Variable lengths via an extra DONE label D per group (46 labels on device):
E'[D,:45] = E[STOP,:], E'[D,D] = 1.0 (exact in bf16), column D otherwise 0.
Active steps emit 0 for D so P[D] stays exactly 0; the absorb step at t=len[b]
emits onehot(D), capturing LSE_j(T[STOP,j]+alpha_j) -- the final answer -- into
P[D]; later steps emit onehot(D) again, multiplying P[D] by exactly 1.0.

Shrinking-width steps: sequences are dealt longest-first round-robin across the
16 (core, group) slots and sorted descending into columns, and the per-step
instruction width n_t is derived from the actual length distribution (exact,
so no sequence outlives its column; a host fallback still guards arbitrary
inputs).  The state lives in ONE in-place tile; step t only updates columns
[0, n_t), so dead columns keep their DONE value frozen.

Per-core critical path per step: one bf16 matmul [92,92]x[92,n_t] with the
stationary blockdiag(E'^T,E'^T) kept loaded in the PE array (standalone
ldweights + non-self-loading matmuls), then one DVE tensor_mul.
"""

import numpy as np
import ml_dtypes

import concourse.bacc as bacc
import concourse.mybir as mybir
import concourse.tile as tile
from concourse.bass_utils import run_bass_kernel_spmd

L = 45
START = 43
STOP = 44
LD = 46                    # labels + DONE landing pad
DONE = 45
B = 1024
S = 512
NCORES = 8
BPC = B // NCORES          # 128 sequences per core
NG = 2                     # groups per core
WCOL = BPC // NG           # 64 columns per group
PR = NG * LD               # 92 partition rows for packed state
TSTEPS = S + 1             # +1 appended absorb step
NSLOTS = NCORES * NG       # 16 (core, group) slots

F32 = mybir.dt.float32
BF16 = mybir.dt.bfloat16
NP_BF16 = ml_dtypes.bfloat16


class _Plan:
    """Width schedule + g layout + chunking, derived from the lengths."""

    def __init__(self, n_sched):
        n_sched = np.asarray(n_sched, np.int64)
        assert n_sched.shape == (S,) and n_sched[0] == WCOL
        self.n_sched = n_sched
        # Column lifetime: last step that still updates column k.
        self.t_col = np.array(
            [
                int((np.where(n_sched > k)[0] + 1).max()) if (n_sched > k).any() else 0
                for k in range(WCOL)
            ],
            np.int64,
        )
        # Per-step g block widths (block 0 is the full-width init state).
        self.blk_w = np.concatenate([[WCOL], n_sched])          # [TSTEPS]
        self.blk_off = np.concatenate([[0], np.cumsum(self.blk_w)])
        self.gcols = int(self.blk_off[-1])
        # Chunk boundaries (step indices): small leading chunks start the
        # pipeline fast; later chunks rotate through a 2-buffer pool with
        # lazily issued DMAs so the transfers trail compute.
        self.chunk_steps = [0, 9, 41] + list(np.linspace(41, TSTEPS, 9).astype(int)[1:])
        self.nchunk = len(self.chunk_steps) - 1
        self.neager = 2
        self.rot_w = int(
            max(
                self.blk_off[self.chunk_steps[c + 1]] - self.blk_off[self.chunk_steps[c]]
                for c in range(self.neager, self.nchunk)
            )
        )
        self.key = n_sched.tobytes()


def _build_nc(plan):
    # Bacc (not raw Bass): its legalization splits multi-sem waits into
    # standalone event-semaphore instructions, which walrus codegen requires.
    nc = bacc.Bacc("TRN2", target_bir_lowering=False, debug=False, num_devices=NCORES)
    # The stationary e2t matrix rides as the first PR columns of g, so one
    # DMA (and one semaphore) gates both the ldweights and the first matmul.
    g_dram = nc.dram_tensor("g", [PR, PR + plan.gcols], BF16, kind="ExternalInput")
    wout_dram = nc.dram_tensor("wout", [PR, WCOL], BF16, kind="ExternalOutput")

    cs = plan.chunk_steps
    with tile.TileContext(nc) as tc:
        with (
            tc.tile_pool(name="geager", bufs=1) as ge_pool,
            tc.tile_pool(name="grot", bufs=2) as gr_pool,
            tc.tile_pool(name="state", bufs=1) as state_pool,
            tc.tile_pool(name="ps_s", bufs=3, space="PSUM") as ps_s,
        ):
            # Chunk 0 carries [e2t | W_0 init | steps 1..8]; chunk 1 the next
            # steps.  Both load immediately on the sync DMA queue.
            gtiles = []
            for c in range(plan.neager):
                c0 = PR + int(plan.blk_off[cs[c]]) if c > 0 else 0
                c1 = PR + int(plan.blk_off[cs[c + 1]])
                gt = ge_pool.tile([PR, c1 - c0], BF16, tag=f"g{c}")
                nc.sync.dma_start(gt[:], g_dram[:, c0:c1])
                gtiles.append(gt)

            e2t = gtiles[0][:, 0:PR]

            # Load blockdiag(E'^T, E'^T) into the PE array once; every step
            # matmul below reuses it (redundant auto-ldweights are stripped
            # after tile legalization below).
            nc.tensor.ldweights(e2t)

            # In-place state: step 1 is full width, so the state tile is
            # fully written by the first tensor_mul; the first matmul's
            # moving operand is the host-folded W_0 block of g directly.
            w_state = state_pool.tile([PR, WCOL], BF16, tag="w")

            chunk_of = np.searchsorted(cs, np.arange(TSTEPS), "right") - 1
            next_chunk = plan.neager
            for t in range(1, TSTEPS):
                # Issue each rotating chunk's DMA ~16 steps before its first
                # use; the 2-buf pool WAR dep keeps transfers trailing compute.
                while next_chunk < plan.nchunk and t >= cs[next_chunk] - 16:
                    cc = next_chunk
                    c0 = PR + int(plan.blk_off[cs[cc]])
                    c1 = PR + int(plan.blk_off[cs[cc + 1]])
                    gt = gr_pool.tile([PR, plan.rot_w], BF16, tag="grot")
                    nc.sync.dma_start(gt[:, 0 : c1 - c0], g_dram[:, c0:c1])
                    gtiles.append(gt)
                    next_chunk += 1
                n = int(plan.n_sched[t - 1])
                c = int(chunk_of[t])
                moving = (
                    gtiles[0][:, PR : PR + WCOL] if t == 1 else w_state[:, 0:n]
                )
                s_ps = ps_s.tile([PR, WCOL], F32, tag="s")
                nc.tensor.matmul(s_ps[:, 0:n], e2t, moving, start=True, stop=True)
                off = int(plan.blk_off[t] - plan.blk_off[cs[c]])
                if c == 0:
                    off += PR
                nc.vector.tensor_mul(
                    w_state[:, 0:n], gtiles[c][:, off : off + n], s_ps[:, 0:n]
                )

            nc.sync.dma_start(wout_dram[:], w_state[:])

    # Tile legalization splits every bf16 matmult into LDWEIGHTS + MATMULT.
    # All those loads are of the SAME stationary tile, so keep only the
    # first (the explicit one above) and drop the rest.  The auto-inserted
    # loads carry no semaphore waits/updates (all sync lives on the
    # matmults), so removal is sync-neutral.
    kept_first = False
    for blk in nc.main_func.blocks:
        for i in list(blk.instructions):
            if isinstance(i, mybir.InstLdweights):
                if not kept_first:
                    kept_first = True
                elif i.sync_info is None:
                    blk.instructions.remove(i)

    nc.compile()
    return nc


_NC_CACHE = {}


def _get_nc():
    """Return the nc built for the most recent _prep_inputs call."""
    return _NC_CACHE["nc"]


def _host_norm(logit_b, len_b, T):
    """Exact float64 log-space forward for one sequence (fallback path)."""
    NEG_INF = -10000.0
    alpha = np.full(L, NEG_INF)
    alpha[START] = 0.0
    for t in range(len_b):
        mat = T + alpha[None, :]
        mx = mat.max(axis=1)
        alpha = logit_b[t] + np.log(np.exp(mat - mx[:, None]).sum(axis=1)) + mx
    v = alpha + T[STOP]
    mx = v.max()
    return np.log(np.exp(v - mx).sum()) + mx


def _prep_inputs(logits, lens, transitions):
    """Host-side preprocessing: exp + absorb-rewrite + deterministic
    per-(seq,step) scaling + length-sorted packing.  Derives the width
    schedule from the actual lengths, builds (or reuses) the matching nc,
    and stashes everything _postprocess needs."""
    logits = np.asarray(logits, np.float32)
    lens = np.asarray(lens, np.int64)
    T = np.asarray(transitions, np.float64)

    E = np.exp(T)                      # [45,45] float64
    erow = E.mean(axis=1)              # mean_j E[i,j], [45]

    Eg = np.zeros((LD, LD), np.float64)
    Eg[:L, :L] = E
    Eg[DONE, :L] = E[STOP, :]
    Eg[DONE, DONE] = 1.0
    e2t = np.zeros((PR, PR), np.float64)
    e2t[:LD, :LD] = Eg.T
    e2t[LD:, LD:] = Eg.T

    G = np.exp(logits.astype(np.float64))          # [B,S,45]

    t_idx = np.arange(S)[None, :]                  # [1,S]
    active = t_idx < lens[:, None]                 # [B,S]

    # Fold step 0 and normalize it exactly: W0 = G0*E[:,START], scale 1/sum.
    W0 = G[:, 0, :] * E[:, START][None, :]         # [B,45]
    m0 = W0.sum(axis=1)                            # [B]
    G[:, 0, :] = W0 / m0[:, None]

    # Active steps t>=1: scale by 1/m_t, m_t = sum_i G_t[i]*erow[i].
    m = G @ erow                                   # [B,S]
    scale_mask = active & (t_idx > 0)
    np.divide(G, m[:, :, None], out=G, where=scale_mask[:, :, None])

    # log-scale accumulator: z[b] = log m0 + sum_{1<=t<len} log m_t.
    logm = np.where(scale_mask, np.log(m), 0.0)
    z = np.log(m0) + logm.sum(axis=1)

    # 46-label emissions: D gets 0 while active, onehot(D) from t>=len on.
    G46 = np.zeros((B, TSTEPS, LD), np.float64)
    G46[:, :S, :L] = np.where(active[:, :, None], G, 0.0)
    done_from = t_idx >= lens[:, None]             # includes absorb step
    G46[:, :S, DONE] = np.where(done_from, 1.0, 0.0)
    G46[:, S, DONE] = 1.0                          # appended step

    # Deal longest-first round-robin across the 16 (core, group) slots.
    order = np.argsort(-lens, kind="stable")
    slots = np.empty((NSLOTS, WCOL), np.int64)
    for r, b in enumerate(order):
        slots[r % NSLOTS][r // NSLOTS] = b
    slot_lens = lens[slots]                        # [NSLOTS, WCOL] descending

    # Exact width schedule: step t updates max-over-slots active columns
    # (clamped so step 1 is full width and the schedule is positive).
    steps = np.arange(1, TSTEPS)                   # [S]
    n_sched = (slot_lens[:, :, None] >= steps[None, None, :]).sum(axis=1).max(axis=0)
    n_sched = np.maximum(n_sched, 1)
    n_sched[0] = WCOL
    n_sched = np.maximum.accumulate(n_sched[::-1])[::-1]  # non-increasing
    plan = _Plan(n_sched)

    if _NC_CACHE.get("key") != plan.key:
        _NC_CACHE["nc"] = _build_nc(plan)
        _NC_CACHE["key"] = plan.key

    # Host fallback for any sequence outliving its column (none when the
    # schedule is derived from these lens, but guards future reuse).
    host_norms = {}
    logits64 = logits.astype(np.float64)
    for s in range(NSLOTS):
        for k in range(WCOL):
            b = slots[s][k]
            if lens[b] > plan.t_col[k]:
                host_norms[int(b)] = _host_norm(logits64[b], int(lens[b]), T)

    _NC_CACHE["plan"] = plan
    _NC_CACHE["z"] = z
    _NC_CACHE["slots"] = slots
    _NC_CACHE["host_norms"] = host_norms

    g16 = G46.astype(NP_BF16)
    e2t16 = e2t.astype(NP_BF16)
    in_maps = []
    for c in range(NCORES):
        g_in = np.zeros((PR, PR + plan.gcols), NP_BF16)
        g_in[:, :PR] = e2t16
        for g in range(NG):
            seqs = slots[c * NG + g]               # [WCOL] original indices
            rows = slice(g * LD, (g + 1) * LD)
            # Per-step blocks: step t occupies cols PR + [blk_off[t], +width).
            gc = g16[seqs]                         # [WCOL, TSTEPS, LD]
            for t in range(TSTEPS):
                w = int(plan.blk_w[t])
                o = PR + int(plan.blk_off[t])
                g_in[rows, o : o + w] = gc[:w, t, :].T
        in_maps.append({"g": g_in})
    return in_maps


def _postprocess(results, lens, transitions):
    z = _NC_CACHE["z"]
    slots = _NC_CACHE["slots"]
    host_norms = _NC_CACHE["host_norms"]
    norm = np.empty(B, np.float64)
    for c in range(NCORES):
        wout = np.asarray(results[c]["wout"]).astype(np.float64)  # [PR, WCOL]
        for g in range(NG):
            seqs = slots[c * NG + g]
            pdone = wout[g * LD + DONE, :]
            norm[seqs] = np.log(pdone) + z[seqs]
    for b, v in host_norms.items():
        norm[b] = v
    return norm.astype(np.float32)


def kernel(logits, lens, transitions):
    in_maps = _prep_inputs(logits, lens, transitions)
    nc = _get_nc()
    res = run_bass_kernel_spmd(nc, in_maps, list(range(NCORES)))
    return _postprocess(res.results, lens, transitions)


# revision 25
# speedup vs baseline: 1.1247x; 1.0619x over previous
"""CRF forward (log partition) on 8 NeuronCores, data-parallel over batch.

Math: the forward recurrence runs in probability space: with E = exp(T) and
G_t = exp(emissions_t), alpha_{t+1} = logit_t + LSE_j(T + alpha_t) becomes the
linear recurrence P_{t+1} = G_t o (E @ P_t).

All normalization is folded into the DATA on the host: each active step's
emission row is pre-scaled by 1/m_t[b] with m_t[b] = sum_i G[b,t,i]*rowmean(E)_i
(a deterministic per-sequence scalar), which keeps the state O(1) in bf16 range
without any data-dependent renorm on device.  The log-scales are accumulated in
float64 host-side and added back at the end.

Variable lengths via an extra DONE label D per group (46 labels on device):
E'[D,:45] = E[STOP,:], E'[D,D] = 1.0 (exact in bf16), column D otherwise 0.
Active steps emit 0 for D so P[D] stays exactly 0; the absorb step at t=len[b]
emits onehot(D), capturing LSE_j(T[STOP,j]+alpha_j) -- the final answer -- into
P[D]; later steps emit onehot(D) again, multiplying P[D] by exactly 1.0.

TWO staggered chains per core: the per-step serial latency is
~400ns fixed + ~1.8ns/column, so each core's 64 columns are split into two
independent chains (even sorted ranks -> device cols 0..31, odd -> 32..63).
Each chain's link is ~1.8ns/col cheaper at half the width, and the chains
interleave on the PE/DVE queues (both engines have idle slack), cutting wall
time below the single-chain floor.  Widths shrink with a per-chain schedule
derived from the actual length distribution; dead columns freeze in the
in-place per-chain state tiles.

Per-core critical path per chain step: one bf16 matmul [92,92]x[92,n] with the
stationary blockdiag(E'^T,E'^T) kept loaded in the PE array (standalone
ldweights + stripped auto-loads), then one DVE tensor_mul.
"""

import numpy as np
import ml_dtypes

import concourse.bacc as bacc
import concourse.mybir as mybir
import concourse.tile as tile
from concourse.bass_utils import run_bass_kernel_spmd

L = 45
START = 43
STOP = 44
LD = 46                    # labels + DONE landing pad
DONE = 45
B = 1024
S = 512
NCORES = 8
BPC = B // NCORES          # 128 sequences per core
NG = 2                     # groups per core
WCOL = BPC // NG           # 64 columns per group
HALF = WCOL // 2           # 32 columns per chain
PR = NG * LD               # 92 partition rows for packed state
TSTEPS = S + 1             # +1 appended absorb step
NSLOTS = NCORES * NG       # 16 (core, group) slots

F32 = mybir.dt.float32
BF16 = mybir.dt.bfloat16
NP_BF16 = ml_dtypes.bfloat16


class _Plan:
    """Dual-chain width schedules + g layout + chunking."""

    def __init__(self, na, nb):
        na = np.asarray(na, np.int64)
        nb = np.asarray(nb, np.int64)
        assert na.shape == (S,) and nb.shape == (S,)
        assert na[0] == HALF and nb[0] == HALF
        self.na, self.nb = na, nb
        # Lifetime (last updated step) per device column.
        self.t_col = np.zeros(WCOL, np.int64)
        for j in range(HALF):
            self.t_col[j] = int((np.where(na > j)[0] + 1).max()) if (na > j).any() else 0
            self.t_col[HALF + j] = (
                int((np.where(nb > j)[0] + 1).max()) if (nb > j).any() else 0
            )
        # Per-step g block widths (block 0 = full-width init, [A|B] layout).
        self.blk_w = np.concatenate([[WCOL], na + nb])          # [TSTEPS]
        self.blk_off = np.concatenate([[0], np.cumsum(self.blk_w)])
        self.gcols = int(self.blk_off[-1])
        # Chunk boundaries (step indices): small leading chunks start the
        # pipeline fast; later chunks rotate through a 2-buffer pool with
        # lazily issued DMAs so the transfers trail compute.
        self.chunk_steps = [0, 9, 41] + list(np.linspace(41, TSTEPS, 9).astype(int)[1:])
        self.nchunk = len(self.chunk_steps) - 1
        self.neager = 2
        self.rot_w = int(
            max(
                self.blk_off[self.chunk_steps[c + 1]] - self.blk_off[self.chunk_steps[c]]
                for c in range(self.neager, self.nchunk)
            )
        )
        self.key = na.tobytes() + nb.tobytes()


def _build_nc(plan):
    # Bacc (not raw Bass): its legalization splits multi-sem waits into
    # standalone event-semaphore instructions, which walrus codegen requires.
    nc = bacc.Bacc("TRN2", target_bir_lowering=False, debug=False, num_devices=NCORES)
    # The stationary e2t matrix rides as the first PR columns of g, so one
    # DMA (and one semaphore) gates both the ldweights and the first matmuls.
    g_dram = nc.dram_tensor("g", [PR, PR + plan.gcols], BF16, kind="ExternalInput")
    wout_dram = nc.dram_tensor("wout", [PR, WCOL], BF16, kind="ExternalOutput")

    cs = plan.chunk_steps
    with tile.TileContext(nc) as tc:
        with (
            tc.tile_pool(name="geager", bufs=1) as ge_pool,
            tc.tile_pool(name="grot", bufs=2) as gr_pool,
            tc.tile_pool(name="state", bufs=1) as state_pool,
            tc.tile_pool(name="ps_s", bufs=3, space="PSUM") as ps_s,
        ):
            gtiles = []
            for c in range(plan.neager):
                c0 = PR + int(plan.blk_off[cs[c]]) if c > 0 else 0
                c1 = PR + int(plan.blk_off[cs[c + 1]])
                gt = ge_pool.tile([PR, c1 - c0], BF16, tag=f"g{c}")
                nc.sync.dma_start(gt[:], g_dram[:, c0:c1])
                gtiles.append(gt)

            e2t = gtiles[0][:, 0:PR]

            # Load blockdiag(E'^T, E'^T) into the PE array once; every step
            # matmul below reuses it (redundant auto-ldweights are stripped
            # after tile legalization below).
            nc.tensor.ldweights(e2t)

            # Per-chain in-place states; step 1 is full width per chain, so
            # both tiles are fully written by the first tensor_muls (the
            # first matmuls read the host-folded W_0 block of g directly).
            w_a = state_pool.tile([PR, HALF], BF16, tag="wa")
            w_b = state_pool.tile([PR, HALF], BF16, tag="wb")

            chunk_of = np.searchsorted(cs, np.arange(TSTEPS), "right") - 1
            next_chunk = plan.neager
            for t in range(1, TSTEPS):
                # Issue each rotating chunk's DMA ~16 steps ahead of use; the
                # 2-buf pool WAR dep keeps transfers trailing compute.
                while next_chunk < plan.nchunk and t >= cs[next_chunk] - 16:
                    cc = next_chunk
                    c0 = PR + int(plan.blk_off[cs[cc]])
                    c1 = PR + int(plan.blk_off[cs[cc + 1]])
                    gt = gr_pool.tile([PR, plan.rot_w], BF16, tag="grot")
                    nc.sync.dma_start(gt[:, 0 : c1 - c0], g_dram[:, c0:c1])
                    gtiles.append(gt)
                    next_chunk += 1
                na = int(plan.na[t - 1])
                nb = int(plan.nb[t - 1])
                c = int(chunk_of[t])
                off = int(plan.blk_off[t] - plan.blk_off[cs[c]])
                if c == 0:
                    off += PR
                gt = gtiles[c]
                mova = gtiles[0][:, PR : PR + HALF] if t == 1 else w_a[:, 0:na]
                movb = (
                    gtiles[0][:, PR + HALF : PR + WCOL] if t == 1 else w_b[:, 0:nb]
                )
                ps_a = ps_s.tile([PR, HALF], F32, tag="sa")
                nc.tensor.matmul(ps_a[:, 0:na], e2t, mova, start=True, stop=True)
                if nb > 0:
                    ps_b = ps_s.tile([PR, HALF], F32, tag="sb")
                    nc.tensor.matmul(ps_b[:, 0:nb], e2t, movb, start=True, stop=True)
                nc.vector.tensor_mul(
                    w_a[:, 0:na], gt[:, off : off + na], ps_a[:, 0:na]
                )
                if nb > 0:
                    nc.vector.tensor_mul(
                        w_b[:, 0:nb], gt[:, off + na : off + na + nb], ps_b[:, 0:nb]
                    )

            nc.sync.dma_start(wout_dram[:, 0:HALF], w_a[:])
            nc.sync.dma_start(wout_dram[:, HALF:WCOL], w_b[:])

    # Tile legalization splits every bf16 matmult into LDWEIGHTS + MATMULT.
    # All those loads are of the SAME stationary tile, so keep only the
    # first (the explicit one above) and drop the rest.  The auto-inserted
    # loads carry no semaphore waits/updates (all sync lives on the
    # matmults), so removal is sync-neutral.
    kept_first = False
    for blk in nc.main_func.blocks:
        for i in list(blk.instructions):
            if isinstance(i, mybir.InstLdweights):
                if not kept_first:
                    kept_first = True
                elif i.sync_info is None:
                    blk.instructions.remove(i)

    nc.compile()
    return nc


_NC_CACHE = {}


def _get_nc():
    """Return the nc built for the most recent _prep_inputs call."""
    return _NC_CACHE["nc"]


def _host_norm(logit_b, len_b, T):
    """Exact float64 log-space forward for one sequence (fallback path)."""
    NEG_INF = -10000.0
    alpha = np.full(L, NEG_INF)
    alpha[START] = 0.0
    for t in range(len_b):
        mat = T + alpha[None, :]
        mx = mat.max(axis=1)
        alpha = logit_b[t] + np.log(np.exp(mat - mx[:, None]).sum(axis=1)) + mx
    v = alpha + T[STOP]
    mx = v.max()
    return np.log(np.exp(v - mx).sum()) + mx


def _prep_inputs(logits, lens, transitions):
    """Host-side preprocessing: exp + absorb-rewrite + deterministic
    per-(seq,step) scaling + dual-chain length-sorted packing."""
    logits = np.asarray(logits, np.float32)
    lens = np.asarray(lens, np.int64)
    T = np.asarray(transitions, np.float64)

    E = np.exp(T)                      # [45,45] float64
    erow = E.mean(axis=1)              # mean_j E[i,j], [45]

    Eg = np.zeros((LD, LD), np.float64)
    Eg[:L, :L] = E
    Eg[DONE, :L] = E[STOP, :]
    Eg[DONE, DONE] = 1.0
    e2t = np.zeros((PR, PR), np.float64)
    e2t[:LD, :LD] = Eg.T
    e2t[LD:, LD:] = Eg.T

    G = np.exp(logits.astype(np.float64))          # [B,S,45]

    t_idx = np.arange(S)[None, :]                  # [1,S]
    active = t_idx < lens[:, None]                 # [B,S]

    # Fold step 0 and normalize it exactly: W0 = G0*E[:,START], scale 1/sum.
    W0 = G[:, 0, :] * E[:, START][None, :]         # [B,45]
    m0 = W0.sum(axis=1)                            # [B]
    G[:, 0, :] = W0 / m0[:, None]

    # Active steps t>=1: scale by 1/m_t, m_t = sum_i G_t[i]*erow[i].
    m = G @ erow                                   # [B,S]
    scale_mask = active & (t_idx > 0)
    np.divide(G, m[:, :, None], out=G, where=scale_mask[:, :, None])

    # log-scale accumulator: z[b] = log m0 + sum_{1<=t<len} log m_t.
    logm = np.where(scale_mask, np.log(m), 0.0)
    z = np.log(m0) + logm.sum(axis=1)

    # 46-label emissions: D gets 0 while active, onehot(D) from t>=len on.
    G46 = np.zeros((B, TSTEPS, LD), np.float64)
    G46[:, :S, :L] = np.where(active[:, :, None], G, 0.0)
    done_from = t_idx >= lens[:, None]             # includes absorb step
    G46[:, :S, DONE] = np.where(done_from, 1.0, 0.0)
    G46[:, S, DONE] = 1.0                          # appended step

    # Deal longest-first round-robin across the 16 (core, group) slots, then
    # split each slot's sorted ranks into chain A (even) / chain B (odd):
    # device cols 0..31 hold ranks 0,2,..,62 and cols 32..63 ranks 1,3,..,63.
    order = np.argsort(-lens, kind="stable")
    slots = np.empty((NSLOTS, WCOL), np.int64)
    for r, b in enumerate(order):
        slots[r % NSLOTS][r // NSLOTS] = b
    perm = np.concatenate([np.arange(0, WCOL, 2), np.arange(1, WCOL, 2)])
    slots_dev = slots[:, perm]                     # [NSLOTS, WCOL] device order
    lens_dev = lens[slots_dev]

    # Exact per-chain width schedules (max over slots), step 1 full width.
    steps = np.arange(1, TSTEPS)                   # [S]
    na = (lens_dev[:, :HALF, None] >= steps[None, None, :]).sum(axis=1).max(axis=0)
    nb = (lens_dev[:, HALF:, None] >= steps[None, None, :]).sum(axis=1).max(axis=0)
    na = np.maximum(na, 1)
    na[0] = HALF
    nb[0] = HALF
    na = np.maximum.accumulate(na[::-1])[::-1]
    nb = np.maximum.accumulate(nb[::-1])[::-1]
    plan = _Plan(na, nb)

    if _NC_CACHE.get("key") != plan.key:
        _NC_CACHE["nc"] = _build_nc(plan)
        _NC_CACHE["key"] = plan.key

    # Host fallback for any sequence outliving its device column (none when
    # the schedule is derived from these lens, but guards arbitrary inputs).
    host_norms = {}
    logits64 = logits.astype(np.float64)
    for s in range(NSLOTS):
        for k in range(WCOL):
            b = slots_dev[s][k]
            if lens[b] > plan.t_col[k]:
                host_norms[int(b)] = _host_norm(logits64[b], int(lens[b]), T)

    _NC_CACHE["plan"] = plan
    _NC_CACHE["z"] = z
    _NC_CACHE["slots_dev"] = slots_dev
    _NC_CACHE["host_norms"] = host_norms

    g16 = G46.astype(NP_BF16)
    e2t16 = e2t.astype(NP_BF16)
    in_maps = []
    for c in range(NCORES):
        g_in = np.zeros((PR, PR + plan.gcols), NP_BF16)
        g_in[:, :PR] = e2t16
        for g in range(NG):
            seqs = slots_dev[c * NG + g]           # [WCOL] device col -> seq
            rows = slice(g * LD, (g + 1) * LD)
            gc = g16[seqs]                         # [WCOL, TSTEPS, LD]
            # Init block: [A cols 0..31 | B cols 32..63], full width.
            g_in[rows, PR : PR + WCOL] = gc[:, 0, :].T
            for t in range(1, TSTEPS):
                wa = int(plan.na[t - 1])
                wb = int(plan.nb[t - 1])
                o = PR + int(plan.blk_off[t])
                g_in[rows, o : o + wa] = gc[:wa, t, :].T
                g_in[rows, o + wa : o + wa + wb] = gc[HALF : HALF + wb, t, :].T
        in_maps.append({"g": g_in})
    return in_maps


def _postprocess(results, lens, transitions):
    z = _NC_CACHE["z"]
    slots_dev = _NC_CACHE["slots_dev"]
    host_norms = _NC_CACHE["host_norms"]
    norm = np.empty(B, np.float64)
    for c in range(NCORES):
        wout = np.asarray(results[c]["wout"]).astype(np.float64)  # [PR, WCOL]
        for g in range(NG):
            seqs = slots_dev[c * NG + g]
            pdone = wout[g * LD + DONE, :]
            norm[seqs] = np.log(pdone) + z[seqs]
    for b, v in host_norms.items():
        norm[b] = v
    return norm.astype(np.float32)


def kernel(logits, lens, transitions):
    in_maps = _prep_inputs(logits, lens, transitions)
    nc = _get_nc()
    res = run_bass_kernel_spmd(nc, in_maps, list(range(NCORES)))
    return _postprocess(res.results, lens, transitions)
